# revision 2
# baseline (speedup 1.0000x reference)
"""GAT encoder on 8 trn2 cores — src-sharded edges + ReduceScatter partials.

Strategy:
 - Nodes are permuted within each core's section (20 blocks x 125 real + 3
   pad rows), LPT-balanced by in-degree so every global dst block receives
   ~2000 edges (<= 2048).
 - Edges are processed by the core owning their SRC node. For layer 1,
   per-(core,block) overflow beyond 256 edges is moved to under-loaded cores
   with the src row replicated there (halo, ~500 rows/core), giving a
   uniform 2 tiles per (core,block) cell: 320 tiles, 40960 slots (2% pad).
   Layer 2 keeps src-owner assignment with 3 tiles/cell (no halo possible
   for projected features).
 - Phase A projects only own+halo nodes (1/8 of the baseline's replicated
   work); only the tiny per-node dst logits are AllGathered (16B/node,
   bf16 hi/lo pairs).
 - Per-edge dst logits come from a transposed one-hot matmul on the PE
   against an SBUF-resident logit table (no 256B/edge DMA gather).
 - Aggregation per dst block via one-hot matmuls into PSUM; partials
   (payload + denominator hi/lo) land in a [20480, 264] bf16 table split in
   two block-halves; a ReduceScatter(add) per half hands each core its own
   fully-reduced rows. The first RS overlaps the second half of each edge
   phase. Replaces the baseline's 330us serialized feature-AllGather chain.
Outputs (mu, logvar) assembled host-side (un-permuted) from per-core slices.
"""

import numpy as np

# ---- problem constants ----
N = 20000
E = 320000
FIN = 512
HID = 256
LAT = 128
H = 4
C1 = 64
NEG = 0.2
EPS = 1e-16

NC = 8
NOWN = 2500
SEC = 2560               # padded section rows (20 blocks)
NBLK = NC * 20           # 160 global dst blocks
RPB = 125                # real nodes per block
AUG = 3584               # local src table rows (2560 own + 1024 halo)
XW = 384                 # physical row width of gather tables (768B)
CW = 264                 # used row width / partial table width

TPB1 = 2                 # L1 tiles per cell
TILES1 = NBLK * TPB1     # 320
SLOTS1 = TILES1 * 128    # 40960
TPB2 = 3                 # L2 tiles per cell
TILES2 = NBLK * TPB2     # 480
SLOTS2 = TILES2 * 128    # 61440
TPC = 16                 # tiles per chunk
CHUNK = TPC * 128        # 2048
IC = CHUNK // 16         # idx cols per chunk
NCH1 = TILES1 // TPC     # 20
NCH2 = TILES2 // TPC     # 30
HROWS = 5 * 128          # rows per quarter-table section
NQ = 4                   # ReduceScatter splits

_cache = {}


def _wrap_idxs(idx):
    n = idx.shape[0]
    t = np.zeros((128, n // 16), np.int16)
    w = idx.reshape(n // 16, 16).T.astype(np.int16)
    for g in range(8):
        t[g * 16:(g + 1) * 16, :] = w
    return t


def _colmajor(a, tiles):
    return np.ascontiguousarray(a.reshape(tiles, 128).T)


def _rowmajor_tiles(a, tiles):
    # per-tile rows for the transposed one-hot build: [128, ceil(T/128), 128]
    reps = (tiles + 127) // 128
    out = np.zeros((128, reps, 128), a.dtype)
    ar = a.reshape(tiles, 128)
    for t in range(tiles):
        out[t % 128, t // 128, :] = ar[t]
    return out


def _build_module(upto=4):
    import concourse.bacc as bacc
    import concourse.mybir as mybir
    import concourse.tile as tile

    f32 = mybir.dt.float32
    bf16 = mybir.dt.bfloat16
    i16 = mybir.dt.int16
    Alu = mybir.AluOpType
    Act = mybir.ActivationFunctionType

    nc = bacc.Bacc("TRN2", target_bir_lowering=False, num_devices=NC,
                   dynamic_dma_scratch_size=65536)

    # ---- inputs ----
    xTb = nc.dram_tensor("xTb", [FIN, AUG], bf16, kind="ExternalInput")
    w1e = nc.dram_tensor("w1e", [FIN, 264], bf16, kind="ExternalInput")
    wmue = nc.dram_tensor("wmue", [HID, 130], bf16, kind="ExternalInput")
    wlve = nc.dram_tensor("wlve", [HID, 130], bf16, kind="ExternalInput")
    b1b = nc.dram_tensor("b1b", [128, 256], f32, kind="ExternalInput")
    bmub = nc.dram_tensor("bmub", [128, 128], f32, kind="ExternalInput")
    blvb = nc.dram_tensor("blvb", [128, 128], f32, kind="ExternalInput")
    iota = nc.dram_tensor("iota", [128, 128], bf16, kind="ExternalInput")
    ident = nc.dram_tensor("ident", [128, 128], f32, kind="ExternalInput")
    srcg1 = nc.dram_tensor("srcg1", [128, SLOTS1 // 16], i16,
                           kind="ExternalInput")
    srcg2 = nc.dram_tensor("srcg2", [128, SLOTS2 // 16], i16,
                           kind="ExternalInput")
    dstoffT1 = nc.dram_tensor("dstoffT1", [128, TILES1], f32,
                              kind="ExternalInput")
    dstoffT2 = nc.dram_tensor("dstoffT2", [128, TILES2], f32,
                              kind="ExternalInput")
    dstl1 = nc.dram_tensor("dstl1", [128, SLOTS1 // 16], i16,
                           kind="ExternalInput")
    dstl2 = nc.dram_tensor("dstl2", [128, SLOTS2 // 16], i16,
                           kind="ExternalInput")
    wT1 = nc.dram_tensor("wT1", [128, TILES1], f32, kind="ExternalInput")
    wT2 = nc.dram_tensor("wT2", [128, TILES2], f32, kind="ExternalInput")

    mu_out = nc.dram_tensor("mu_out", [SEC, LAT], f32, kind="ExternalOutput")
    lv_out = nc.dram_tensor("lv_out", [SEC, LAT], f32, kind="ExternalOutput")

    with tile.TileContext(nc) as tc:
        with (
            tc.tile_pool(name="cst", bufs=1) as cst,
            tc.tile_pool(name="lw", bufs=3) as lw,
            tc.tile_pool(name="xa", bufs=3) as xa,
            tc.tile_pool(name="gx", bufs=3) as gx,
            tc.tile_pool(name="oh", bufs=40) as ohp,
            tc.tile_pool(name="sm", bufs=6) as sm,
            tc.tile_pool(name="fin", bufs=3) as fin,
            tc.tile_pool(name="ps2", bufs=3, space="PSUM") as ps2,
            tc.tile_pool(name="psa", bufs=1, space="PSUM") as psa,
            tc.tile_pool(name="ps1", bufs=1, space="PSUM") as ps1,
            tc.tile_pool(name="ge", bufs=3) as ge,
            tc.tile_pool(name="dr", bufs=1, space="DRAM") as dr,
        ):
            XPT = dr.tile([AUG, XW], bf16, tag="XPT")
            MLT = dr.tile([SEC, XW], bf16, tag="MLT")
            sd1in = dr.tile([SEC, 8], bf16, tag="sd1in")
            SD1T = dr.tile([NC * SEC // 16, 128], bf16, tag="SD1T",
                           addr_space="Shared")
            SDW1 = dr.tile([NC * SEC, 128], bf16, tag="SDW1")
            sd2in = dr.tile([SEC, 8], bf16, tag="sd2in")
            SD2T = dr.tile([NC * SEC // 16, 128], bf16, tag="SD2T",
                           addr_space="Shared")
            SDW2 = dr.tile([NC * SEC, 128], bf16, tag="SDW2")
            TBL1 = []
            TBL2 = []
            RS1O = []
            RS2O = []
            for hf in range(NQ):
                TBL1.append(dr.tile([NC * HROWS, CW], bf16, tag=f"TBL1{hf}",
                                    name=f"TBL1{hf}"))
                TBL2.append(dr.tile([NC * HROWS, CW], bf16, tag=f"TBL2{hf}",
                                    name=f"TBL2{hf}"))
                RS1O.append(dr.tile([HROWS, CW], bf16, tag=f"RS1O{hf}",
                                    name=f"RS1O{hf}"))
                RS2O.append(dr.tile([HROWS, CW], bf16, tag=f"RS2O{hf}",
                                    name=f"RS2O{hf}"))

            # resident constants
            def cload(shape, dtype, tag, srcap):
                t = cst.tile(shape, dtype, tag=tag)
                nc.sync.dma_start(t[:], srcap)
                return t

            w1e_t = [cload([128, 264], bf16, f"w1e{kk}",
                           w1e[kk * 128:(kk + 1) * 128, :]) for kk in range(4)]

            # ---- phase A: own groups, then AG1, then halo group ----
            def phase_a_group(g):
                lx = lw.tile([128, 4, 512], bf16, tag="lx")
                nc.sync.dma_start(
                    lx[:], xTb[:].rearrange("(kk p) (g n) -> p kk g n",
                                            p=128, n=512)[:, :, g, :])
                xps = xa.tile([128, 4, 272], bf16, tag="xps")
                sdh = xa.tile([128, 4, 8], bf16, tag="sdh")
                for ti in range(4):
                    ps = psa.tile([128, 264], f32, tag="psA", name="psA",
                                  bufs=2)
                    for kk in range(4):
                        sl = slice(ti * 128, (ti + 1) * 128)
                        nc.tensor.matmul(ps[:], lx[:, kk, sl], w1e_t[kk][:],
                                         start=(kk == 0), stop=(kk == 3))
                    nc.scalar.copy(xps[:, ti, 0:256], ps[:, 0:256])
                    # ss as f32 in slots 256:264
                    nc.vector.tensor_copy(
                        xps[:, ti, 256:272].bitcast(f32), ps[:, 256:264])
                    # sd hi/lo bf16 pairs for the logit AllGather
                    nc.vector.tensor_copy(sdh[:, ti, 0:4], ps[:, 260:264])
                    nc.vector.tensor_tensor(
                        sdh[:, ti, 4:8], ps[:, 260:264], sdh[:, ti, 0:4],
                        op=Alu.subtract)
                nc.sync.dma_start(
                    XPT[:].rearrange("(g4 p) c -> p g4 c", p=128)
                    [:, 4 * g:4 * g + 4, 0:264], xps[:, :, 0:264])
                if g < SEC // 512:
                    nc.sync.dma_start(
                        sd1in[:].rearrange("(g4 p) c -> p g4 c", p=128)
                        [:, 4 * g:4 * g + 4, :], sdh[:])

            for g in range(SEC // 512):
                phase_a_group(g)
            srcg1_t = cload([128, SLOTS1 // 16], i16, "srcg1", srcg1[:])
            dstl1_t = cload([128, SLOTS1 // 16], i16, "dstl1", dstl1[:])
            dstoffT1_t = cload([128, TILES1], f32, "dstoffT1", dstoffT1[:])
            wT1_t = cload([128, TILES1], f32, "wT1", wT1[:])
            iota_t = cload([128, 128], bf16, "iota", iota[:])
            wmue_t = [cload([128, 130], bf16, f"wmue{kk}",
                            wmue[kk * 128:(kk + 1) * 128, :])
                      for kk in range(2)]
            wlve_t = [cload([128, 130], bf16, f"wlve{kk}",
                            wlve[kk * 128:(kk + 1) * 128, :])
                      for kk in range(2)]
            b1b_t = cload([128, 256], f32, "b1b", b1b[:])
            bmub_t = cload([128, 128], f32, "bmub", bmub[:])
            blvb_t = cload([128, 128], f32, "blvb", blvb[:])
            ident_t = cload([128, 128], f32, "ident", ident[:])
            nc.gpsimd.collective_compute(
                "AllGather", mybir.AluOpType.bypass,
                replica_groups=[list(range(NC))],
                ins=[sd1in[:]], outs=[SD1T[:]])
            for s16 in range(16):
                nc.scalar.dma_start(
                    SDW1[:].rearrange("(r s) c -> r s c", s=16)
                    [:, s16, 0:8],
                    SD1T[:, s16 * 8:(s16 + 1) * 8])
            for g in range(SEC // 512, AUG // 512):
                phase_a_group(g)

            def edge_phase(layer, SRC_TBL, SDT_TBL, srcg_t, dofT, dstl_t,
                           wTt, TBLh, nchunk, tpb, rs_cb):
                nh = 4 if layer == 1 else 2
                blk_ps = {}
                ext = None
                qe = [((q + 1) * 40 * tpb) // TPC + 4 for q in range(3)]
                for ci in range(nchunk):
                    for q in range(3):
                        if ci == qe[q]:
                            rs_cb(q)
                    ohx_t = {}
                    for tt in range(TPC):
                        t = ci * TPC + tt
                        ohx = ohp.tile([128, 128], bf16, tag="ohx")
                        eng = nc.vector if tt % 4 != 3 else nc.gpsimd
                        eng.tensor_scalar(
                            ohx[:], iota_t[:], dofT[:, t:t + 1], None,
                            Alu.is_equal)
                        ohx_t[tt] = ohx
                    xrow = gx.tile([128, TPC, XW], bf16, tag="xrow")
                    HT = TPC // 2
                    HIC = IC // 2
                    for gh in range(2):
                        nc.gpsimd.dma_gather(
                            xrow[:, gh * HT:(gh + 1) * HT, :], SRC_TBL[:],
                            srcg_t[:, ci * IC + gh * HIC:
                                   ci * IC + (gh + 1) * HIC],
                            CHUNK // 2, CHUNK // 2, XW)
                    ext = ge.tile([128, TPC, 128], bf16, tag="ext")
                    for gh in range(2):
                        nc.gpsimd.dma_gather(
                            ext[:, gh * HT:(gh + 1) * HT, :], SDT_TBL[:],
                            dstl_t[:, ci * IC + gh * HIC:
                                   ci * IC + (gh + 1) * HIC],
                            CHUNK // 2, CHUNK // 2, 128)
                    exs = ext[:, :, 0:8]
                    z = sm.tile([128, TPC, nh], f32, tag="z")
                    if layer == 1:
                        nc.vector.tensor_tensor(
                            z[:], xrow[:, :, 256:264].bitcast(f32),
                            exs[:, :, 0:4], op=Alu.add)
                        nc.vector.tensor_tensor(
                            z[:], z[:], exs[:, :, 4:8], op=Alu.add)
                    else:
                        nc.vector.tensor_tensor(
                            z[:],
                            xrow[:, :, 256:264].bitcast(f32)[:, :, 0:2],
                            exs[:, :, 0:2], op=Alu.add)
                        nc.vector.tensor_tensor(
                            z[:], z[:], exs[:, :, 4:6], op=Alu.add)
                    nc.vector.scalar_tensor_tensor(
                        z[:], in0=z[:], scalar=NEG, in1=z[:],
                        op0=Alu.mult, op1=Alu.max)
                    ex = sm.tile([128, TPC, nh], f32, tag="ex")
                    nc.scalar.activation(ex[:], z[:], Act.Exp)
                    exw = sm.tile([128, TPC, nh], f32, tag="exw")
                    wb = wTt[:, ci * TPC:(ci + 1) * TPC]
                    nc.vector.tensor_tensor(
                        exw[:], ex[:],
                        wb.rearrange("p (t o) -> p t o", o=1).to_broadcast(
                            [128, TPC, nh]), op=Alu.mult)
                    exw2 = sm.tile([128, TPC, nh, 2], bf16, tag="exw2")
                    nc.vector.tensor_copy(
                        exw2[:], exw[:].rearrange("p t (h o) -> p t h o", o=1)
                        .to_broadcast([128, TPC, nh, 2]))
                    kw = 256 // nh // 2
                    xrh = xrow[:, :, 0:256].rearrange(
                        "p t (h k two) -> p t h k two", h=nh, two=2)
                    nc.vector.tensor_tensor(
                        xrh, xrh,
                        exw2[:].rearrange("p t h (o two) -> p t h o two",
                                          two=2)
                        .to_broadcast([128, TPC, nh, kw, 2]), op=Alu.mult)
                    nc.vector.tensor_copy(xrow[:, :, 256:256 + nh], ex[:])
                    nc.vector.tensor_tensor(
                        xrow[:, :, 256 + nh:256 + 2 * nh], ex[:],
                        xrow[:, :, 256:256 + nh], op=Alu.subtract)

                    for tt in range(TPC):
                        t = ci * TPC + tt
                        cell = t // tpb
                        k = t % tpb
                        if k == 0:
                            blk_ps[cell] = ps2.tile([128, 264], f32,
                                                    tag="blk", name="blkps",
                                                    bufs=4)
                        ps = blk_ps[cell]
                        nc.tensor.matmul(
                            ps[:, 0:264], ohx_t[tt][:], xrow[:, tt, 0:264],
                            start=(k == 0), stop=(k == tpb - 1))
                        if k == tpb - 1:
                            hf = cell // 40
                            rb = cell % 40
                            cpy = fin.tile([128, 264], bf16, tag="cpy",
                                           bufs=8)
                            nc.scalar.copy(cpy[:], ps[:, 0:264])
                            nc.sync.dma_start(
                                TBLh[hf][rb * 128:(rb + 1) * 128, :], cpy[:])
                            del blk_ps[cell]

            def emit_rs(TBLh, RSOh, hf):
                nc.gpsimd.collective_compute(
                    "ReduceScatter", Alu.add,
                    replica_groups=[list(range(NC))],
                    ins=[TBLh[hf][:]], outs=[RSOh[hf][:]])

            srcg2_t = cload([128, SLOTS2 // 16], i16, "srcg2", srcg2[:])
            dstoffT2_t = cload([128, TILES2], f32, "dstoffT2", dstoffT2[:])
            dstl2_t = cload([128, SLOTS2 // 16], i16, "dstl2", dstl2[:])
            wT2_t = cload([128, TILES2], f32, "wT2", wT2[:])
            # ---- L1 ----
            if upto >= 2:
                edge_phase(1, XPT, SDW1, srcg1_t, dstoffT1_t, dstl1_t, wT1_t,
                           TBL1, NCH1, TPB1,
                           (lambda hf: emit_rs(TBL1, RS1O, hf))
                           if upto >= 3 else (lambda hf: None))
            if upto >= 3:
                emit_rs(TBL1, RS1O, 3)

            # ---- L1 finalize ----
            for b in range(20 if upto >= 3 else 0):
                hf = b // 5
                rbase = (b % 5) * 128
                rsb = fin.tile([128, 264], bf16, tag="rsb")
                nc.sync.dma_start(rsb[:], RS1O[hf][rbase:rbase + 128, :])
                den = sm.tile([128, 4], f32, tag="den")
                nc.vector.tensor_tensor(den[:], rsb[:, 256:260],
                                        rsb[:, 260:264], op=Alu.add)
                nc.vector.tensor_scalar_add(den[:], den[:], EPS)
                rec = sm.tile([128, 4], f32, tag="rec")
                nc.vector.reciprocal(rec[:], den[:])
                hb = fin.tile([128, 256], f32, tag="hb")
                for h in range(H):
                    nc.vector.scalar_tensor_tensor(
                        hb[:, h * 64:(h + 1) * 64],
                        in0=rsb[:, h * 64:(h + 1) * 64],
                        scalar=rec[:, h:h + 1],
                        in1=b1b_t[:, h * 64:(h + 1) * 64],
                        op0=Alu.mult, op1=Alu.add)
                zm = fin.tile([128, 256], f32, tag="zm")
                nc.vector.tensor_scalar_min(zm[:], hb[:], 0.0)
                ez = fin.tile([128, 256], f32, tag="ez")
                nc.scalar.activation(ez[:], zm[:], Act.Exp)
                nc.vector.scalar_tensor_tensor(
                    hb[:], in0=hb[:], scalar=0.0, in1=ez[:],
                    op0=Alu.max, op1=Alu.add)
                nc.vector.tensor_scalar_add(hb[:], hb[:], -1.0)
                hTs = []
                for half in range(2):
                    pst = psa.tile([128, 264], f32, tag="psA", name="pstA",
                                   bufs=2)
                    nc.tensor.transpose(
                        pst[:, 0:128], hb[:, half * 128:(half + 1) * 128],
                        ident_t[:])
                    hT = fin.tile([128, 128], bf16, tag=f"hT{half}")
                    nc.vector.tensor_copy(hT[:], pst[:, 0:128])
                    hTs.append(hT)
                psmu_t = ps1.tile([128, 130], f32, tag="psmu", name="psmu")
                pslv_t = ps1.tile([128, 130], f32, tag="pslv", name="pslv")
                psmu = psmu_t[:]
                pslv = pslv_t[:]
                for kk in range(2):
                    nc.tensor.matmul(psmu, hTs[kk][:], wmue_t[kk][:],
                                     start=(kk == 0), stop=(kk == 1))
                    nc.tensor.matmul(pslv, hTs[kk][:], wlve_t[kk][:],
                                     start=(kk == 0), stop=(kk == 1))
                xr2 = fin.tile([128, 264], bf16, tag="xr2")
                nc.scalar.copy(xr2[:, 0:128], psmu[:, 0:128])
                nc.scalar.copy(xr2[:, 128:256], pslv[:, 0:128])
                # ss as f32 slots [ssmu, sslv]; sd hi/lo bf16 for AG2
                ssv = xr2[:, 256:264].bitcast(f32)
                nc.vector.tensor_copy(ssv[:, 0:1], psmu[:, 128:129])
                nc.vector.tensor_copy(ssv[:, 1:2], pslv[:, 128:129])
                sdh = fin.tile([128, 8], bf16, tag="sdh2")
                nc.vector.tensor_copy(sdh[:, 0:1], psmu[:, 129:130])
                nc.vector.tensor_copy(sdh[:, 1:2], pslv[:, 129:130])
                nc.vector.tensor_tensor(sdh[:, 4:5], psmu[:, 129:130],
                                        sdh[:, 0:1], op=Alu.subtract)
                nc.vector.tensor_tensor(sdh[:, 5:6], pslv[:, 129:130],
                                        sdh[:, 1:2], op=Alu.subtract)
                nc.sync.dma_start(MLT[b * 128:(b + 1) * 128, 0:264], xr2[:])
                nc.sync.dma_start(sd2in[b * 128:(b + 1) * 128, :], sdh[:])

            if upto >= 3:
                nc.gpsimd.collective_compute(
                    "AllGather", mybir.AluOpType.bypass,
                    replica_groups=[list(range(NC))],
                    ins=[sd2in[:]], outs=[SD2T[:]])
                for s16 in range(16):
                    nc.scalar.dma_start(
                        SDW2[:].rearrange("(r s) c -> r s c", s=16)
                        [:, s16, 0:8],
                        SD2T[:, s16 * 8:(s16 + 1) * 8])

            # ---- L2/3 ----
            if upto >= 4:
                edge_phase(2, MLT, SDW2, srcg2_t, dstoffT2_t, dstl2_t, wT2_t,
                           TBL2, NCH2, TPB2,
                           lambda hf: emit_rs(TBL2, RS2O, hf))
                emit_rs(TBL2, RS2O, 3)

            # ---- final ----
            for b in range(20 if upto >= 4 else 1):
                hf = b // 5
                rbase = (b % 5) * 128
                rsb = fin.tile([128, 264], bf16, tag="rsb2")
                nc.sync.dma_start(rsb[:], RS2O[hf][rbase:rbase + 128, :])
                for li, (bias_t, outdr) in enumerate(
                        ((bmub_t, mu_out), (blvb_t, lv_out))):
                    den = sm.tile([128, 1], f32, tag="den2")
                    nc.vector.tensor_tensor(
                        den[:], rsb[:, 256 + li:257 + li],
                        rsb[:, 258 + li:259 + li], op=Alu.add)
                    nc.vector.tensor_scalar_add(den[:], den[:], EPS)
                    rec = sm.tile([128, 1], f32, tag="rec2")
                    nc.vector.reciprocal(rec[:], den[:])
                    ob = fin.tile([128, 128], f32, tag="ob")
                    nc.vector.scalar_tensor_tensor(
                        ob[:], in0=rsb[:, li * 128:(li + 1) * 128],
                        scalar=rec[:, 0:1],
                        in1=bias_t[:], op0=Alu.mult, op1=Alu.add)
                    nc.sync.dma_start(
                        outdr[b * 128:(b + 1) * 128, :], ob[:])

    nc.compile()
    return nc


def _prep_inputs(x, edge_index, edge_weight, W1, att1, b1, Wmu, attmu, bmu,
                 Wlv, attlv, blv):
    import ml_dtypes
    bf = ml_dtypes.bfloat16

    src = np.asarray(edge_index[0], np.int64)
    dst = np.asarray(edge_index[1], np.int64)
    w = np.asarray(edge_weight, np.float32)
    x = np.asarray(x, np.float32)

    att1 = np.asarray(att1, np.float32)
    W1 = np.asarray(W1, np.float32)
    Wss1 = np.zeros((FIN, H), np.float32)
    Wsd1 = np.zeros((FIN, H), np.float32)
    for h in range(H):
        Wss1[:, h] = W1[:, h * C1:(h + 1) * C1] @ att1[h, C1:]
        Wsd1[:, h] = W1[:, h * C1:(h + 1) * C1] @ att1[h, :C1]
    w1e = np.concatenate([W1, Wss1, Wsd1], axis=1).astype(bf)

    attmu = np.asarray(attmu, np.float32).reshape(-1)
    attlv = np.asarray(attlv, np.float32).reshape(-1)
    Wmu = np.asarray(Wmu, np.float32)
    Wlv = np.asarray(Wlv, np.float32)
    wmue = np.concatenate(
        [Wmu, (Wmu @ attmu[LAT:])[:, None], (Wmu @ attmu[:LAT])[:, None]],
        axis=1).astype(bf)
    wlve = np.concatenate(
        [Wlv, (Wlv @ attlv[LAT:])[:, None], (Wlv @ attlv[:LAT])[:, None]],
        axis=1).astype(bf)

    xT_all = x.T.astype(bf)
    b1b = np.tile(np.asarray(b1, np.float32)[None, :], (128, 1))
    bmub = np.tile(np.asarray(bmu, np.float32)[None, :], (128, 1))
    blvb = np.tile(np.asarray(blv, np.float32)[None, :], (128, 1))
    iota = np.tile(np.arange(128, dtype=np.float32)[None, :],
                   (128, 1)).astype(bf)
    iotaP = np.ascontiguousarray(iota.T)
    ident = np.eye(128, dtype=np.float32)

    # ---- node permutation: LPT balance blocks by in-degree ----
    deg = np.bincount(dst, minlength=N).astype(np.int64)
    permrow = np.zeros(N, np.int64)     # node -> global padded row
    inv_rows = np.full(NC * SEC, -1, np.int64)  # row -> node
    for c in range(NC):
        nodes = np.arange(c * NOWN, (c + 1) * NOWN)
        order = nodes[np.argsort(-deg[nodes], kind="stable")]
        loads = np.zeros(20, np.int64)
        fill = np.zeros(20, np.int64)
        for nd in order:
            cand = np.where(fill < RPB)[0]
            bsel = cand[np.argmin(loads[cand])]
            permrow[nd] = c * SEC + bsel * 128 + fill[bsel]
            inv_rows[permrow[nd]] = nd
            loads[bsel] += deg[nd]
            fill[bsel] += 1

    prow = permrow[dst]
    gblk = prow // 128
    goff = prow % 128
    blktot = np.bincount(gblk, minlength=NBLK)
    assert blktot.max() <= NC * 256, f"block overflow {blktot.max()}"

    score = src // NOWN
    # L1 cell rebalancing: cap 256 per (core, block), move with halo
    assign = score.copy()
    eidx_by_cell = {}
    for b in range(NBLK):
        eb = np.where(gblk == b)[0]
        cores = assign[eb]
        cnt = np.bincount(cores, minlength=NC)
        over_c = [c for c in range(NC) if cnt[c] > 256]
        space = {c: 256 - int(cnt[c]) for c in range(NC)}
        movers = []
        for c in over_c:
            ec = eb[cores == c]
            movers.extend(ec[256:].tolist())
            space[c] = 0
        ptr = 0
        order = sorted(space, key=lambda k: -space[k])
        while ptr < len(movers):
            moved = False
            for c in order:
                if space[c] > 0 and ptr < len(movers):
                    assign[movers[ptr]] = c
                    space[c] -= 1
                    ptr += 1
                    moved = True
            assert moved
        assert ptr == len(movers)

    # local row maps: own nodes at their permuted slot, halo appended
    q_of = (gblk % 20) // 5
    cell_of = q_of * 40 + (gblk // 20) * 5 + (gblk % 20) % 5

    in_maps = []
    for c in range(NC):
        m1 = assign == c
        e1 = np.where(m1)[0]
        own = score[e1] == c
        halo_nodes = np.unique(src[e1[~own]])
        assert SEC + len(halo_nodes) <= AUG, len(halo_nodes)
        lrow = np.full(N, -1, np.int64)
        sec_nodes = inv_rows[c * SEC:(c + 1) * SEC]
        valid = sec_nodes >= 0
        lrow[sec_nodes[valid]] = np.where(valid)[0]
        lrow[halo_nodes] = SEC + np.arange(len(halo_nodes))

        def pack(eids, tpb, tiles, slots):
            e_src = np.zeros(slots, np.int64)
            e_dst = np.zeros(slots, np.int64)
            e_off = np.full(slots, -1.0, np.float32)
            e_w = np.zeros(slots, np.float32)
            cells = cell_of[eids]
            for cell in range(160):
                bm = cells == cell
                ee = eids[bm]
                nbe = len(ee)
                assert nbe <= tpb * 128, (c, cell, nbe)
                o = cell * tpb * 128
                e_src[o:o + nbe] = lrow[src[ee]]
                e_dst[o:o + nbe] = prow[ee]
                e_off[o:o + nbe] = goff[ee].astype(np.float32)
                e_w[o:o + nbe] = w[ee]
            assert e_src.min() >= 0
            return e_src, e_dst, e_off, e_w

        s1, d1, o1, w1 = pack(e1, TPB1, TILES1, SLOTS1)
        e2 = np.where(score == c)[0]
        s2, d2, o2, w2 = pack(e2, TPB2, TILES2, SLOTS2)

        xTb_c = np.zeros((FIN, AUG), bf)
        cols = sec_nodes.copy()
        ok = cols >= 0
        xTb_c[:, :SEC][:, ok] = xT_all[:, cols[ok]]
        xTb_c[:, SEC:SEC + len(halo_nodes)] = xT_all[:, halo_nodes]

        in_maps.append({
            "xTb": xTb_c, "w1e": w1e, "wmue": wmue, "wlve": wlve, "b1b": b1b,
            "bmub": bmub, "blvb": blvb, "iota": iota, "iotaP": iotaP,
            "ident": ident,
            "srcg1": _wrap_idxs(s1), "srcg2": _wrap_idxs(s2),
            "dstoffT1": _colmajor(o1, TILES1),
            "dstoffT2": _colmajor(o2, TILES2),
            "dstl1": _wrap_idxs(d1), "dstl2": _wrap_idxs(d2),
            "wT1": _colmajor(w1, TILES1), "wT2": _colmajor(w2, TILES2),
        })
    return in_maps, inv_rows


def kernel(x, edge_index, edge_weight, W1, att1, b1, Wmu, attmu, bmu,
           Wlv, attlv, blv):
    from concourse.bass_utils import run_bass_kernel_spmd

    if "nc" not in _cache:
        _cache["nc"] = _build_module()
    nc = _cache["nc"]
    in_maps, inv_rows = _prep_inputs(x, edge_index, edge_weight, W1, att1, b1,
                                     Wmu, attmu, bmu, Wlv, attlv, blv)
    r = run_bass_kernel_spmd(nc, in_maps, list(range(NC)))
    mu = np.zeros((N, LAT), np.float32)
    lv = np.zeros((N, LAT), np.float32)
    for c in range(NC):
        rows = inv_rows[c * SEC:(c + 1) * SEC]
        ok = rows >= 0
        mu[rows[ok]] = r.results[c]["mu_out"][ok]
        lv[rows[ok]] = r.results[c]["lv_out"][ok]
    return (mu, lv)


# revision 8
# speedup vs baseline: 1.0065x; 1.0065x over previous
"""GAT encoder on 8 trn2 cores — src-sharded edges + ReduceScatter partials.

Strategy:
 - Nodes are permuted within each core's section (20 blocks x 125 real + 3
   pad rows), LPT-balanced by in-degree so every global dst block receives
   ~2000 edges (<= 2048).
 - Edges are processed by the core owning their SRC node. For layer 1,
   per-(core,block) overflow beyond 256 edges is moved to under-loaded cores
   with the src row replicated there (halo, ~500 rows/core), giving a
   uniform 2 tiles per (core,block) cell: 320 tiles, 40960 slots (2% pad).
   Layer 2 keeps src-owner assignment with 3 tiles/cell (no halo possible
   for projected features).
 - Phase A projects only own+halo nodes (1/8 of the baseline's replicated
   work); only the tiny per-node dst logits are AllGathered (16B/node,
   bf16 hi/lo pairs).
 - Per-edge dst logits come from a transposed one-hot matmul on the PE
   against an SBUF-resident logit table (no 256B/edge DMA gather).
 - Aggregation per dst block via one-hot matmuls into PSUM; partials
   (payload + denominator hi/lo) land in a [20480, 264] bf16 table split in
   two block-halves; a ReduceScatter(add) per half hands each core its own
   fully-reduced rows. The first RS overlaps the second half of each edge
   phase. Replaces the baseline's 330us serialized feature-AllGather chain.
Outputs (mu, logvar) assembled host-side (un-permuted) from per-core slices.
"""

import numpy as np

# ---- problem constants ----
N = 20000
E = 320000
FIN = 512
HID = 256
LAT = 128
H = 4
C1 = 64
NEG = 0.2
EPS = 1e-16

NC = 8
NOWN = 2500
SEC = 2560               # padded section rows (20 blocks)
NBLK = NC * 20           # 160 global dst blocks
RPB = 125                # real nodes per block
AUG = 3584               # local src table rows (2560 own + 1024 halo)
XW = 384                 # physical row width of gather tables (768B)
CW = 264                 # used row width / partial table width

TPB1 = 2                 # L1 tiles per cell
TILES1 = NBLK * TPB1     # 320
SLOTS1 = TILES1 * 128    # 40960
TPB2 = 3                 # L2 tiles per cell
TILES2 = NBLK * TPB2     # 480
SLOTS2 = TILES2 * 128    # 61440
TPC = 16                 # tiles per chunk
CHUNK = TPC * 128        # 2048
IC = CHUNK // 16         # idx cols per chunk
NCH1 = TILES1 // TPC     # 20
NCH2 = TILES2 // TPC     # 30
HROWS = 5 * 128          # rows per quarter-table section
NQ = 4                   # ReduceScatter splits

_cache = {}


def _wrap_idxs(idx):
    n = idx.shape[0]
    t = np.zeros((128, n // 16), np.int16)
    w = idx.reshape(n // 16, 16).T.astype(np.int16)
    for g in range(8):
        t[g * 16:(g + 1) * 16, :] = w
    return t


def _colmajor(a, tiles):
    return np.ascontiguousarray(a.reshape(tiles, 128).T)


def _rowmajor_tiles(a, tiles):
    # per-tile rows for the transposed one-hot build: [128, ceil(T/128), 128]
    reps = (tiles + 127) // 128
    out = np.zeros((128, reps, 128), a.dtype)
    ar = a.reshape(tiles, 128)
    for t in range(tiles):
        out[t % 128, t // 128, :] = ar[t]
    return out


def _build_module(upto=4):
    import concourse.bacc as bacc
    import concourse.mybir as mybir
    import concourse.tile as tile

    f32 = mybir.dt.float32
    bf16 = mybir.dt.bfloat16
    i16 = mybir.dt.int16
    Alu = mybir.AluOpType
    Act = mybir.ActivationFunctionType

    nc = bacc.Bacc("TRN2", target_bir_lowering=False, num_devices=NC,
                   dynamic_dma_scratch_size=65536)

    # ---- inputs ----
    xTb = nc.dram_tensor("xTb", [FIN, AUG], bf16, kind="ExternalInput")
    w1e = nc.dram_tensor("w1e", [FIN, 264], bf16, kind="ExternalInput")
    wmue = nc.dram_tensor("wmue", [HID, 130], bf16, kind="ExternalInput")
    wlve = nc.dram_tensor("wlve", [HID, 130], bf16, kind="ExternalInput")
    b1b = nc.dram_tensor("b1b", [128, 256], f32, kind="ExternalInput")
    bmub = nc.dram_tensor("bmub", [128, 128], f32, kind="ExternalInput")
    blvb = nc.dram_tensor("blvb", [128, 128], f32, kind="ExternalInput")
    iota = nc.dram_tensor("iota", [128, 128], bf16, kind="ExternalInput")
    ident = nc.dram_tensor("ident", [128, 128], f32, kind="ExternalInput")
    srcg1 = nc.dram_tensor("srcg1", [128, SLOTS1 // 16], i16,
                           kind="ExternalInput")
    srcg2 = nc.dram_tensor("srcg2", [128, SLOTS2 // 16], i16,
                           kind="ExternalInput")
    dstoffT1 = nc.dram_tensor("dstoffT1", [128, TILES1], f32,
                              kind="ExternalInput")
    dstoffT2 = nc.dram_tensor("dstoffT2", [128, TILES2], f32,
                              kind="ExternalInput")
    dstl1 = nc.dram_tensor("dstl1", [128, SLOTS1 // 16], i16,
                           kind="ExternalInput")
    dstl2 = nc.dram_tensor("dstl2", [128, SLOTS2 // 16], i16,
                           kind="ExternalInput")
    wT1 = nc.dram_tensor("wT1", [128, TILES1], f32, kind="ExternalInput")
    wT2 = nc.dram_tensor("wT2", [128, TILES2], f32, kind="ExternalInput")

    mu_out = nc.dram_tensor("mu_out", [SEC, LAT], f32, kind="ExternalOutput")
    lv_out = nc.dram_tensor("lv_out", [SEC, LAT], f32, kind="ExternalOutput")

    with tile.TileContext(nc) as tc:
        with (
            tc.tile_pool(name="cst", bufs=1) as cst,
            tc.tile_pool(name="lw", bufs=3) as lw,
            tc.tile_pool(name="xa", bufs=3) as xa,
            tc.tile_pool(name="gx", bufs=3) as gx,
            tc.tile_pool(name="oh", bufs=40) as ohp,
            tc.tile_pool(name="sm", bufs=6) as sm,
            tc.tile_pool(name="fin", bufs=3) as fin,
            tc.tile_pool(name="ps2", bufs=3, space="PSUM") as ps2,
            tc.tile_pool(name="psa", bufs=1, space="PSUM") as psa,
            tc.tile_pool(name="ps1", bufs=1, space="PSUM") as ps1,
            tc.tile_pool(name="ge", bufs=3) as ge,
            tc.tile_pool(name="dr", bufs=1, space="DRAM") as dr,
        ):
            XPT = dr.tile([AUG, XW], bf16, tag="XPT")
            MLT = dr.tile([SEC, XW], bf16, tag="MLT")
            sd1in = dr.tile([SEC, 8], bf16, tag="sd1in")
            SD1T = dr.tile([NC * SEC // 16, 128], bf16, tag="SD1T",
                           addr_space="Shared")
            SDW1 = dr.tile([NC * SEC, 128], bf16, tag="SDW1")
            sd2in = dr.tile([SEC, 8], bf16, tag="sd2in")
            SD2T = dr.tile([NC * SEC // 16, 128], bf16, tag="SD2T",
                           addr_space="Shared")
            SDW2 = dr.tile([NC * SEC, 128], bf16, tag="SDW2")
            TBL1 = []
            TBL2 = []
            RS1O = []
            RS2O = []
            for hf in range(NQ):
                TBL1.append(dr.tile([NC * HROWS, CW], bf16, tag=f"TBL1{hf}",
                                    name=f"TBL1{hf}"))
                TBL2.append(dr.tile([NC * HROWS, CW], bf16, tag=f"TBL2{hf}",
                                    name=f"TBL2{hf}"))
                RS1O.append(dr.tile([HROWS, CW], bf16, tag=f"RS1O{hf}",
                                    name=f"RS1O{hf}"))
                RS2O.append(dr.tile([HROWS, CW], bf16, tag=f"RS2O{hf}",
                                    name=f"RS2O{hf}"))

            # resident constants
            def cload(shape, dtype, tag, srcap):
                t = cst.tile(shape, dtype, tag=tag)
                nc.sync.dma_start(t[:], srcap)
                return t

            w1e_t = [cload([128, 264], bf16, f"w1e{kk}",
                           w1e[kk * 128:(kk + 1) * 128, :]) for kk in range(4)]

            # ---- phase A: own groups, then AG1, then halo group ----
            def phase_a_group(g):
                lx = lw.tile([128, 4, 512], bf16, tag="lx")
                nc.sync.dma_start(
                    lx[:], xTb[:].rearrange("(kk p) (g n) -> p kk g n",
                                            p=128, n=512)[:, :, g, :])
                xps = xa.tile([128, 4, 272], bf16, tag="xps")
                sdh = xa.tile([128, 4, 8], bf16, tag="sdh")
                for ti in range(4):
                    ps = psa.tile([128, 264], f32, tag="psA", name="psA",
                                  bufs=2)
                    for kk in range(4):
                        sl = slice(ti * 128, (ti + 1) * 128)
                        nc.tensor.matmul(ps[:], lx[:, kk, sl], w1e_t[kk][:],
                                         start=(kk == 0), stop=(kk == 3))
                    nc.scalar.copy(xps[:, ti, 0:256], ps[:, 0:256])
                    # ss as f32 in slots 256:264
                    nc.vector.tensor_copy(
                        xps[:, ti, 256:272].bitcast(f32), ps[:, 256:264])
                    # sd hi/lo bf16 pairs for the logit AllGather
                    nc.vector.tensor_copy(sdh[:, ti, 0:4], ps[:, 260:264])
                    nc.vector.tensor_tensor(
                        sdh[:, ti, 4:8], ps[:, 260:264], sdh[:, ti, 0:4],
                        op=Alu.subtract)
                nc.sync.dma_start(
                    XPT[:].rearrange("(g4 p) c -> p g4 c", p=128)
                    [:, 4 * g:4 * g + 4, 0:264], xps[:, :, 0:264])
                if g < SEC // 512:
                    nc.sync.dma_start(
                        sd1in[:].rearrange("(g4 p) c -> p g4 c", p=128)
                        [:, 4 * g:4 * g + 4, :], sdh[:])

            for g in range(SEC // 512):
                phase_a_group(g)
            srcg1_t = cload([128, SLOTS1 // 16], i16, "srcg1", srcg1[:])
            dstl1_t = cload([128, SLOTS1 // 16], i16, "dstl1", dstl1[:])
            dstoffT1_t = cload([128, TILES1], f32, "dstoffT1", dstoffT1[:])
            wT1_t = cload([128, TILES1], f32, "wT1", wT1[:])
            iota_t = cload([128, 128], bf16, "iota", iota[:])
            wmue_t = [cload([128, 130], bf16, f"wmue{kk}",
                            wmue[kk * 128:(kk + 1) * 128, :])
                      for kk in range(2)]
            wlve_t = [cload([128, 130], bf16, f"wlve{kk}",
                            wlve[kk * 128:(kk + 1) * 128, :])
                      for kk in range(2)]
            b1b_t = cload([128, 256], f32, "b1b", b1b[:])
            bmub_t = cload([128, 128], f32, "bmub", bmub[:])
            blvb_t = cload([128, 128], f32, "blvb", blvb[:])
            ident_t = cload([128, 128], f32, "ident", ident[:])
            nc.gpsimd.collective_compute(
                "AllGather", mybir.AluOpType.bypass,
                replica_groups=[list(range(NC))],
                ins=[sd1in[:]], outs=[SD1T[:]])
            for s16 in range(16):
                nc.scalar.dma_start(
                    SDW1[:].rearrange("(r s) c -> r s c", s=16)
                    [:, s16, 0:8],
                    SD1T[:, s16 * 8:(s16 + 1) * 8])
            for g in range(SEC // 512, AUG // 512):
                phase_a_group(g)

            def edge_phase(layer, SRC_TBL, SDT_TBL, srcg_t, dofT, dstl_t,
                           wTt, TBLh, nchunk, tpb, rs_cb):
                nh = 4 if layer == 1 else 2
                blk_ps = {}
                ext = None
                qe = [min(((q + 1) * 40 * tpb) // TPC + 5, nchunk - 1 - (2 - q))
                      for q in range(3)]
                for ci in range(nchunk):
                    for q in range(3):
                        if ci == qe[q]:
                            rs_cb(q)
                    ohx_t = {}
                    for tt in range(TPC):
                        t = ci * TPC + tt
                        ohx = ohp.tile([128, 128], bf16, tag="ohx")
                        eng = nc.vector if tt % 4 != 3 else nc.gpsimd
                        eng.tensor_scalar(
                            ohx[:], iota_t[:], dofT[:, t:t + 1], None,
                            Alu.is_equal)
                        ohx_t[tt] = ohx
                    xrow = gx.tile([128, TPC, XW], bf16, tag="xrow")
                    HT = TPC // 2
                    HIC = IC // 2
                    for gh in range(2):
                        nc.gpsimd.dma_gather(
                            xrow[:, gh * HT:(gh + 1) * HT, :], SRC_TBL[:],
                            srcg_t[:, ci * IC + gh * HIC:
                                   ci * IC + (gh + 1) * HIC],
                            CHUNK // 2, CHUNK // 2, XW)
                    ext = ge.tile([128, TPC, 128], bf16, tag="ext")
                    for gh in range(2):
                        nc.gpsimd.dma_gather(
                            ext[:, gh * HT:(gh + 1) * HT, :], SDT_TBL[:],
                            dstl_t[:, ci * IC + gh * HIC:
                                   ci * IC + (gh + 1) * HIC],
                            CHUNK // 2, CHUNK // 2, 128)
                    exs = ext[:, :, 0:8]
                    z = sm.tile([128, TPC, nh], f32, tag="z")
                    if layer == 1:
                        nc.vector.tensor_tensor(
                            z[:], xrow[:, :, 256:264].bitcast(f32),
                            exs[:, :, 0:4], op=Alu.add)
                        nc.vector.tensor_tensor(
                            z[:], z[:], exs[:, :, 4:8], op=Alu.add)
                    else:
                        nc.vector.tensor_tensor(
                            z[:],
                            xrow[:, :, 256:264].bitcast(f32)[:, :, 0:2],
                            exs[:, :, 0:2], op=Alu.add)
                        nc.vector.tensor_tensor(
                            z[:], z[:], exs[:, :, 4:6], op=Alu.add)
                    nc.vector.scalar_tensor_tensor(
                        z[:], in0=z[:], scalar=NEG, in1=z[:],
                        op0=Alu.mult, op1=Alu.max)
                    ex = sm.tile([128, TPC, nh], f32, tag="ex")
                    nc.scalar.activation(ex[:], z[:], Act.Exp)
                    exw = sm.tile([128, TPC, nh], f32, tag="exw")
                    wb = wTt[:, ci * TPC:(ci + 1) * TPC]
                    nc.vector.tensor_tensor(
                        exw[:], ex[:],
                        wb.rearrange("p (t o) -> p t o", o=1).to_broadcast(
                            [128, TPC, nh]), op=Alu.mult)
                    exw2 = sm.tile([128, TPC, nh, 2], bf16, tag="exw2")
                    nc.vector.tensor_copy(
                        exw2[:], exw[:].rearrange("p t (h o) -> p t h o", o=1)
                        .to_broadcast([128, TPC, nh, 2]))
                    kw = 256 // nh // 2
                    xrh = xrow[:, :, 0:256].rearrange(
                        "p t (h k two) -> p t h k two", h=nh, two=2)
                    nc.vector.tensor_tensor(
                        xrh, xrh,
                        exw2[:].rearrange("p t h (o two) -> p t h o two",
                                          two=2)
                        .to_broadcast([128, TPC, nh, kw, 2]), op=Alu.mult)
                    nc.vector.tensor_copy(xrow[:, :, 256:256 + nh], ex[:])
                    nc.vector.tensor_tensor(
                        xrow[:, :, 256 + nh:256 + 2 * nh], ex[:],
                        xrow[:, :, 256:256 + nh], op=Alu.subtract)

                    for tt in range(TPC):
                        t = ci * TPC + tt
                        cell = t // tpb
                        k = t % tpb
                        if k == 0:
                            blk_ps[cell] = ps2.tile([128, 264], f32,
                                                    tag="blk", name="blkps",
                                                    bufs=4)
                        ps = blk_ps[cell]
                        nc.tensor.matmul(
                            ps[:, 0:264], ohx_t[tt][:], xrow[:, tt, 0:264],
                            start=(k == 0), stop=(k == tpb - 1))
                        if k == tpb - 1:
                            hf = cell // 40
                            rb = cell % 40
                            cpy = fin.tile([128, 264], bf16, tag="cpy",
                                           bufs=8)
                            nc.scalar.copy(cpy[:], ps[:, 0:264])
                            nc.sync.dma_start(
                                TBLh[hf][rb * 128:(rb + 1) * 128, :], cpy[:])
                            del blk_ps[cell]

            def emit_rs(TBLh, RSOh, hf):
                nc.gpsimd.collective_compute(
                    "ReduceScatter", Alu.add,
                    replica_groups=[list(range(NC))],
                    ins=[TBLh[hf][:]], outs=[RSOh[hf][:]])

            srcg2_t = cload([128, SLOTS2 // 16], i16, "srcg2", srcg2[:])
            dstoffT2_t = cload([128, TILES2], f32, "dstoffT2", dstoffT2[:])
            dstl2_t = cload([128, SLOTS2 // 16], i16, "dstl2", dstl2[:])
            wT2_t = cload([128, TILES2], f32, "wT2", wT2[:])
            # ---- L1 ----
            if upto >= 2:
                edge_phase(1, XPT, SDW1, srcg1_t, dstoffT1_t, dstl1_t, wT1_t,
                           TBL1, NCH1, TPB1,
                           (lambda hf: emit_rs(TBL1, RS1O, hf))
                           if upto >= 3 else (lambda hf: None))
            if upto >= 3:
                emit_rs(TBL1, RS1O, 3)

            # ---- L1 finalize ----
            for b in range(20 if upto >= 3 else 0):
                hf = b // 5
                rbase = (b % 5) * 128
                rsb = fin.tile([128, 264], bf16, tag="rsb")
                nc.sync.dma_start(rsb[:], RS1O[hf][rbase:rbase + 128, :])
                den = sm.tile([128, 4], f32, tag="den")
                nc.vector.tensor_tensor(den[:], rsb[:, 256:260],
                                        rsb[:, 260:264], op=Alu.add)
                nc.vector.tensor_scalar_add(den[:], den[:], EPS)
                rec = sm.tile([128, 4], f32, tag="rec")
                nc.vector.reciprocal(rec[:], den[:])
                hb = fin.tile([128, 256], f32, tag="hb")
                for h in range(H):
                    nc.vector.scalar_tensor_tensor(
                        hb[:, h * 64:(h + 1) * 64],
                        in0=rsb[:, h * 64:(h + 1) * 64],
                        scalar=rec[:, h:h + 1],
                        in1=b1b_t[:, h * 64:(h + 1) * 64],
                        op0=Alu.mult, op1=Alu.add)
                zm = fin.tile([128, 256], f32, tag="zm")
                nc.vector.tensor_scalar_min(zm[:], hb[:], 0.0)
                ez = fin.tile([128, 256], f32, tag="ez")
                nc.scalar.activation(ez[:], zm[:], Act.Exp)
                nc.vector.scalar_tensor_tensor(
                    hb[:], in0=hb[:], scalar=0.0, in1=ez[:],
                    op0=Alu.max, op1=Alu.add)
                nc.vector.tensor_scalar_add(hb[:], hb[:], -1.0)
                hTs = []
                for half in range(2):
                    pst = psa.tile([128, 264], f32, tag="psA", name="pstA",
                                   bufs=2)
                    nc.tensor.transpose(
                        pst[:, 0:128], hb[:, half * 128:(half + 1) * 128],
                        ident_t[:])
                    hT = fin.tile([128, 128], bf16, tag=f"hT{half}")
                    nc.vector.tensor_copy(hT[:], pst[:, 0:128])
                    hTs.append(hT)
                psmu_t = ps1.tile([128, 130], f32, tag="psmu", name="psmu")
                pslv_t = ps1.tile([128, 130], f32, tag="pslv", name="pslv")
                psmu = psmu_t[:]
                pslv = pslv_t[:]
                for kk in range(2):
                    nc.tensor.matmul(psmu, hTs[kk][:], wmue_t[kk][:],
                                     start=(kk == 0), stop=(kk == 1))
                    nc.tensor.matmul(pslv, hTs[kk][:], wlve_t[kk][:],
                                     start=(kk == 0), stop=(kk == 1))
                xr2 = fin.tile([128, 264], bf16, tag="xr2")
                nc.scalar.copy(xr2[:, 0:128], psmu[:, 0:128])
                nc.scalar.copy(xr2[:, 128:256], pslv[:, 0:128])
                # ss as f32 slots [ssmu, sslv]; sd hi/lo bf16 for AG2
                ssv = xr2[:, 256:264].bitcast(f32)
                nc.vector.tensor_copy(ssv[:, 0:1], psmu[:, 128:129])
                nc.vector.tensor_copy(ssv[:, 1:2], pslv[:, 128:129])
                sdh = fin.tile([128, 8], bf16, tag="sdh2")
                nc.vector.tensor_copy(sdh[:, 0:1], psmu[:, 129:130])
                nc.vector.tensor_copy(sdh[:, 1:2], pslv[:, 129:130])
                nc.vector.tensor_tensor(sdh[:, 4:5], psmu[:, 129:130],
                                        sdh[:, 0:1], op=Alu.subtract)
                nc.vector.tensor_tensor(sdh[:, 5:6], pslv[:, 129:130],
                                        sdh[:, 1:2], op=Alu.subtract)
                nc.sync.dma_start(MLT[b * 128:(b + 1) * 128, 0:264], xr2[:])
                nc.sync.dma_start(sd2in[b * 128:(b + 1) * 128, :], sdh[:])

            if upto >= 3:
                nc.gpsimd.collective_compute(
                    "AllGather", mybir.AluOpType.bypass,
                    replica_groups=[list(range(NC))],
                    ins=[sd2in[:]], outs=[SD2T[:]])
                for s16 in range(16):
                    nc.scalar.dma_start(
                        SDW2[:].rearrange("(r s) c -> r s c", s=16)
                        [:, s16, 0:8],
                        SD2T[:, s16 * 8:(s16 + 1) * 8])

            # ---- L2/3 ----
            if upto >= 4:
                edge_phase(2, MLT, SDW2, srcg2_t, dstoffT2_t, dstl2_t, wT2_t,
                           TBL2, NCH2, TPB2,
                           lambda hf: emit_rs(TBL2, RS2O, hf))
                emit_rs(TBL2, RS2O, 3)

            # ---- final ----
            for b in range(20 if upto >= 4 else 1):
                hf = b // 5
                rbase = (b % 5) * 128
                rsb = fin.tile([128, 264], bf16, tag="rsb2")
                nc.sync.dma_start(rsb[:], RS2O[hf][rbase:rbase + 128, :])
                for li, (bias_t, outdr) in enumerate(
                        ((bmub_t, mu_out), (blvb_t, lv_out))):
                    den = sm.tile([128, 1], f32, tag="den2")
                    nc.vector.tensor_tensor(
                        den[:], rsb[:, 256 + li:257 + li],
                        rsb[:, 258 + li:259 + li], op=Alu.add)
                    nc.vector.tensor_scalar_add(den[:], den[:], EPS)
                    rec = sm.tile([128, 1], f32, tag="rec2")
                    nc.vector.reciprocal(rec[:], den[:])
                    ob = fin.tile([128, 128], f32, tag="ob")
                    nc.vector.scalar_tensor_tensor(
                        ob[:], in0=rsb[:, li * 128:(li + 1) * 128],
                        scalar=rec[:, 0:1],
                        in1=bias_t[:], op0=Alu.mult, op1=Alu.add)
                    nc.sync.dma_start(
                        outdr[b * 128:(b + 1) * 128, :], ob[:])

    nc.compile()
    return nc


def _prep_inputs(x, edge_index, edge_weight, W1, att1, b1, Wmu, attmu, bmu,
                 Wlv, attlv, blv):
    import ml_dtypes
    bf = ml_dtypes.bfloat16

    src = np.asarray(edge_index[0], np.int64)
    dst = np.asarray(edge_index[1], np.int64)
    w = np.asarray(edge_weight, np.float32)
    x = np.asarray(x, np.float32)

    att1 = np.asarray(att1, np.float32)
    W1 = np.asarray(W1, np.float32)
    Wss1 = np.zeros((FIN, H), np.float32)
    Wsd1 = np.zeros((FIN, H), np.float32)
    for h in range(H):
        Wss1[:, h] = W1[:, h * C1:(h + 1) * C1] @ att1[h, C1:]
        Wsd1[:, h] = W1[:, h * C1:(h + 1) * C1] @ att1[h, :C1]
    w1e = np.concatenate([W1, Wss1, Wsd1], axis=1).astype(bf)

    attmu = np.asarray(attmu, np.float32).reshape(-1)
    attlv = np.asarray(attlv, np.float32).reshape(-1)
    Wmu = np.asarray(Wmu, np.float32)
    Wlv = np.asarray(Wlv, np.float32)
    wmue = np.concatenate(
        [Wmu, (Wmu @ attmu[LAT:])[:, None], (Wmu @ attmu[:LAT])[:, None]],
        axis=1).astype(bf)
    wlve = np.concatenate(
        [Wlv, (Wlv @ attlv[LAT:])[:, None], (Wlv @ attlv[:LAT])[:, None]],
        axis=1).astype(bf)

    xT_all = x.T.astype(bf)
    b1b = np.tile(np.asarray(b1, np.float32)[None, :], (128, 1))
    bmub = np.tile(np.asarray(bmu, np.float32)[None, :], (128, 1))
    blvb = np.tile(np.asarray(blv, np.float32)[None, :], (128, 1))
    iota = np.tile(np.arange(128, dtype=np.float32)[None, :],
                   (128, 1)).astype(bf)
    iotaP = np.ascontiguousarray(iota.T)
    ident = np.eye(128, dtype=np.float32)

    # ---- node permutation: LPT balance blocks by in-degree ----
    deg = np.bincount(dst, minlength=N).astype(np.int64)
    permrow = np.zeros(N, np.int64)     # node -> global padded row
    inv_rows = np.full(NC * SEC, -1, np.int64)  # row -> node
    for c in range(NC):
        nodes = np.arange(c * NOWN, (c + 1) * NOWN)
        order = nodes[np.argsort(-deg[nodes], kind="stable")]
        loads = np.zeros(20, np.int64)
        fill = np.zeros(20, np.int64)
        for nd in order:
            cand = np.where(fill < RPB)[0]
            bsel = cand[np.argmin(loads[cand])]
            permrow[nd] = c * SEC + bsel * 128 + fill[bsel]
            inv_rows[permrow[nd]] = nd
            loads[bsel] += deg[nd]
            fill[bsel] += 1

    prow = permrow[dst]
    gblk = prow // 128
    goff = prow % 128
    blktot = np.bincount(gblk, minlength=NBLK)
    assert blktot.max() <= NC * 256, f"block overflow {blktot.max()}"

    score = src // NOWN
    # L1 cell rebalancing: cap 256 per (core, block), move with halo
    assign = score.copy()
    eidx_by_cell = {}
    for b in range(NBLK):
        eb = np.where(gblk == b)[0]
        cores = assign[eb]
        cnt = np.bincount(cores, minlength=NC)
        over_c = [c for c in range(NC) if cnt[c] > 256]
        space = {c: 256 - int(cnt[c]) for c in range(NC)}
        movers = []
        for c in over_c:
            ec = eb[cores == c]
            movers.extend(ec[256:].tolist())
            space[c] = 0
        ptr = 0
        order = sorted(space, key=lambda k: -space[k])
        while ptr < len(movers):
            moved = False
            for c in order:
                if space[c] > 0 and ptr < len(movers):
                    assign[movers[ptr]] = c
                    space[c] -= 1
                    ptr += 1
                    moved = True
            assert moved
        assert ptr == len(movers)

    # local row maps: own nodes at their permuted slot, halo appended
    q_of = (gblk % 20) // 5
    cell_of = q_of * 40 + (gblk // 20) * 5 + (gblk % 20) % 5

    in_maps = []
    for c in range(NC):
        m1 = assign == c
        e1 = np.where(m1)[0]
        own = score[e1] == c
        halo_nodes = np.unique(src[e1[~own]])
        assert SEC + len(halo_nodes) <= AUG, len(halo_nodes)
        lrow = np.full(N, -1, np.int64)
        sec_nodes = inv_rows[c * SEC:(c + 1) * SEC]
        valid = sec_nodes >= 0
        lrow[sec_nodes[valid]] = np.where(valid)[0]
        lrow[halo_nodes] = SEC + np.arange(len(halo_nodes))

        def pack(eids, tpb, tiles, slots):
            e_src = np.zeros(slots, np.int64)
            e_dst = np.zeros(slots, np.int64)
            e_off = np.full(slots, -1.0, np.float32)
            e_w = np.zeros(slots, np.float32)
            cells = cell_of[eids]
            for cell in range(160):
                bm = cells == cell
                ee = eids[bm]
                nbe = len(ee)
                assert nbe <= tpb * 128, (c, cell, nbe)
                o = cell * tpb * 128
                e_src[o:o + nbe] = lrow[src[ee]]
                e_dst[o:o + nbe] = prow[ee]
                e_off[o:o + nbe] = goff[ee].astype(np.float32)
                e_w[o:o + nbe] = w[ee]
            assert e_src.min() >= 0
            return e_src, e_dst, e_off, e_w

        s1, d1, o1, w1 = pack(e1, TPB1, TILES1, SLOTS1)
        e2 = np.where(score == c)[0]
        s2, d2, o2, w2 = pack(e2, TPB2, TILES2, SLOTS2)

        xTb_c = np.zeros((FIN, AUG), bf)
        cols = sec_nodes.copy()
        ok = cols >= 0
        xTb_c[:, :SEC][:, ok] = xT_all[:, cols[ok]]
        xTb_c[:, SEC:SEC + len(halo_nodes)] = xT_all[:, halo_nodes]

        in_maps.append({
            "xTb": xTb_c, "w1e": w1e, "wmue": wmue, "wlve": wlve, "b1b": b1b,
            "bmub": bmub, "blvb": blvb, "iota": iota, "iotaP": iotaP,
            "ident": ident,
            "srcg1": _wrap_idxs(s1), "srcg2": _wrap_idxs(s2),
            "dstoffT1": _colmajor(o1, TILES1),
            "dstoffT2": _colmajor(o2, TILES2),
            "dstl1": _wrap_idxs(d1), "dstl2": _wrap_idxs(d2),
            "wT1": _colmajor(w1, TILES1), "wT2": _colmajor(w2, TILES2),
        })
    return in_maps, inv_rows


def kernel(x, edge_index, edge_weight, W1, att1, b1, Wmu, attmu, bmu,
           Wlv, attlv, blv):
    from concourse.bass_utils import run_bass_kernel_spmd

    if "nc" not in _cache:
        _cache["nc"] = _build_module()
    nc = _cache["nc"]
    in_maps, inv_rows = _prep_inputs(x, edge_index, edge_weight, W1, att1, b1,
                                     Wmu, attmu, bmu, Wlv, attlv, blv)
    r = run_bass_kernel_spmd(nc, in_maps, list(range(NC)))
    mu = np.zeros((N, LAT), np.float32)
    lv = np.zeros((N, LAT), np.float32)
    for c in range(NC):
        rows = inv_rows[c * SEC:(c + 1) * SEC]
        ok = rows >= 0
        mu[rows[ok]] = r.results[c]["mu_out"][ok]
        lv[rows[ok]] = r.results[c]["lv_out"][ok]
    return (mu, lv)


# revision 9
# speedup vs baseline: 1.0256x; 1.0190x over previous
"""GAT encoder on 8 trn2 cores — src-sharded edges + ReduceScatter partials.

Strategy:
 - Nodes are permuted within each core's section (20 blocks x 125 real + 3
   pad rows), LPT-balanced by in-degree so every global dst block receives
   ~2000 edges (<= 2048).
 - Edges are processed by the core owning their SRC node. For layer 1,
   per-(core,block) overflow beyond 256 edges is moved to under-loaded cores
   with the src row replicated there (halo, ~500 rows/core), giving a
   uniform 2 tiles per (core,block) cell: 320 tiles, 40960 slots (2% pad).
   Layer 2 keeps src-owner assignment with 3 tiles/cell (no halo possible
   for projected features).
 - Phase A projects only own+halo nodes (1/8 of the baseline's replicated
   work); only the tiny per-node dst logits are AllGathered (16B/node,
   bf16 hi/lo pairs).
 - Per-edge dst logits come from a transposed one-hot matmul on the PE
   against an SBUF-resident logit table (no 256B/edge DMA gather).
 - Aggregation per dst block via one-hot matmuls into PSUM; partials
   (payload + denominator hi/lo) land in a [20480, 264] bf16 table split in
   two block-halves; a ReduceScatter(add) per half hands each core its own
   fully-reduced rows. The first RS overlaps the second half of each edge
   phase. Replaces the baseline's 330us serialized feature-AllGather chain.
Outputs (mu, logvar) assembled host-side (un-permuted) from per-core slices.
"""

import numpy as np

# ---- problem constants ----
N = 20000
E = 320000
FIN = 512
HID = 256
LAT = 128
H = 4
C1 = 64
NEG = 0.2
EPS = 1e-16

NC = 8
NOWN = 2500
SEC = 2560               # padded section rows (20 blocks)
NBLK = NC * 20           # 160 global dst blocks
RPB = 125                # real nodes per block
AUG = 3584               # local src table rows (2560 own + 1024 halo)
XW = 384                 # physical row width of gather tables (768B)
CW = 264                 # used row width / partial table width

TPB1 = 2                 # L1 tiles per cell
TILES1 = NBLK * TPB1     # 320
SLOTS1 = TILES1 * 128    # 40960
TPB2 = 3                 # L2 tiles per cell
TILES2 = NBLK * TPB2     # 480
SLOTS2 = TILES2 * 128    # 61440
TPC = 16                 # tiles per chunk
CHUNK = TPC * 128        # 2048
IC = CHUNK // 16         # idx cols per chunk
NCH1 = TILES1 // TPC     # 20
NCH2 = TILES2 // TPC     # 30
NQ = 4                   # ReduceScatter splits (uneven)
QSIZES = (7, 7, 5, 1)    # blocks per quarter; last tiny to shrink the tail
QLO = (0, 7, 14, 19)     # first block of each quarter
QSTART = (0, 56, 112, 152)  # first cell index of each quarter

_cache = {}


def _wrap_idxs(idx):
    n = idx.shape[0]
    t = np.zeros((128, n // 16), np.int16)
    w = idx.reshape(n // 16, 16).T.astype(np.int16)
    for g in range(8):
        t[g * 16:(g + 1) * 16, :] = w
    return t


def _colmajor(a, tiles):
    return np.ascontiguousarray(a.reshape(tiles, 128).T)


def _rowmajor_tiles(a, tiles):
    # per-tile rows for the transposed one-hot build: [128, ceil(T/128), 128]
    reps = (tiles + 127) // 128
    out = np.zeros((128, reps, 128), a.dtype)
    ar = a.reshape(tiles, 128)
    for t in range(tiles):
        out[t % 128, t // 128, :] = ar[t]
    return out


def _build_module(upto=4):
    import concourse.bacc as bacc
    import concourse.mybir as mybir
    import concourse.tile as tile

    f32 = mybir.dt.float32
    bf16 = mybir.dt.bfloat16
    i16 = mybir.dt.int16
    Alu = mybir.AluOpType
    Act = mybir.ActivationFunctionType

    nc = bacc.Bacc("TRN2", target_bir_lowering=False, num_devices=NC,
                   dynamic_dma_scratch_size=65536)

    # ---- inputs ----
    xTb = nc.dram_tensor("xTb", [FIN, AUG], bf16, kind="ExternalInput")
    w1e = nc.dram_tensor("w1e", [FIN, 264], bf16, kind="ExternalInput")
    wmue = nc.dram_tensor("wmue", [HID, 130], bf16, kind="ExternalInput")
    wlve = nc.dram_tensor("wlve", [HID, 130], bf16, kind="ExternalInput")
    b1b = nc.dram_tensor("b1b", [128, 256], f32, kind="ExternalInput")
    bmub = nc.dram_tensor("bmub", [128, 128], f32, kind="ExternalInput")
    blvb = nc.dram_tensor("blvb", [128, 128], f32, kind="ExternalInput")
    iota = nc.dram_tensor("iota", [128, 128], bf16, kind="ExternalInput")
    ident = nc.dram_tensor("ident", [128, 128], f32, kind="ExternalInput")
    srcg1 = nc.dram_tensor("srcg1", [128, SLOTS1 // 16], i16,
                           kind="ExternalInput")
    srcg2 = nc.dram_tensor("srcg2", [128, SLOTS2 // 16], i16,
                           kind="ExternalInput")
    dstoffT1 = nc.dram_tensor("dstoffT1", [128, TILES1], f32,
                              kind="ExternalInput")
    dstoffT2 = nc.dram_tensor("dstoffT2", [128, TILES2], f32,
                              kind="ExternalInput")
    dstl1 = nc.dram_tensor("dstl1", [128, SLOTS1 // 16], i16,
                           kind="ExternalInput")
    dstl2 = nc.dram_tensor("dstl2", [128, SLOTS2 // 16], i16,
                           kind="ExternalInput")
    wT1 = nc.dram_tensor("wT1", [128, TILES1], f32, kind="ExternalInput")
    wT2 = nc.dram_tensor("wT2", [128, TILES2], f32, kind="ExternalInput")

    mu_out = nc.dram_tensor("mu_out", [SEC, LAT], f32, kind="ExternalOutput")
    lv_out = nc.dram_tensor("lv_out", [SEC, LAT], f32, kind="ExternalOutput")

    with tile.TileContext(nc) as tc:
        with (
            tc.tile_pool(name="cst", bufs=1) as cst,
            tc.tile_pool(name="lw", bufs=3) as lw,
            tc.tile_pool(name="xa", bufs=3) as xa,
            tc.tile_pool(name="gx", bufs=3) as gx,
            tc.tile_pool(name="oh", bufs=40) as ohp,
            tc.tile_pool(name="sm", bufs=6) as sm,
            tc.tile_pool(name="fin", bufs=3) as fin,
            tc.tile_pool(name="ps2", bufs=3, space="PSUM") as ps2,
            tc.tile_pool(name="psa", bufs=1, space="PSUM") as psa,
            tc.tile_pool(name="ps1", bufs=1, space="PSUM") as ps1,
            tc.tile_pool(name="ge", bufs=3) as ge,
            tc.tile_pool(name="dr", bufs=1, space="DRAM") as dr,
        ):
            XPT = dr.tile([AUG, XW], bf16, tag="XPT")
            MLT = dr.tile([SEC, XW], bf16, tag="MLT")
            sd1in = dr.tile([SEC, 8], bf16, tag="sd1in")
            SD1T = dr.tile([NC * SEC // 16, 128], bf16, tag="SD1T",
                           addr_space="Shared")
            SDW1 = dr.tile([NC * SEC, 128], bf16, tag="SDW1")
            sd2in = dr.tile([SEC, 8], bf16, tag="sd2in")
            SD2T = dr.tile([NC * SEC // 16, 128], bf16, tag="SD2T",
                           addr_space="Shared")
            SDW2 = dr.tile([NC * SEC, 128], bf16, tag="SDW2")
            TBL1 = []
            TBL2 = []
            RS1O = []
            RS2O = []
            for hf in range(NQ):
                qr = QSIZES[hf] * 128
                TBL1.append(dr.tile([NC * qr, CW], bf16, tag=f"TBL1{hf}",
                                    name=f"TBL1{hf}"))
                TBL2.append(dr.tile([NC * qr, CW], bf16, tag=f"TBL2{hf}",
                                    name=f"TBL2{hf}"))
                RS1O.append(dr.tile([qr, CW], bf16, tag=f"RS1O{hf}",
                                    name=f"RS1O{hf}"))
                RS2O.append(dr.tile([qr, CW], bf16, tag=f"RS2O{hf}",
                                    name=f"RS2O{hf}"))

            # resident constants
            def cload(shape, dtype, tag, srcap):
                t = cst.tile(shape, dtype, tag=tag)
                nc.sync.dma_start(t[:], srcap)
                return t

            w1e_t = [cload([128, 264], bf16, f"w1e{kk}",
                           w1e[kk * 128:(kk + 1) * 128, :]) for kk in range(4)]

            # ---- phase A: own groups, then AG1, then halo group ----
            def phase_a_group(g):
                lx = lw.tile([128, 4, 512], bf16, tag="lx")
                nc.sync.dma_start(
                    lx[:], xTb[:].rearrange("(kk p) (g n) -> p kk g n",
                                            p=128, n=512)[:, :, g, :])
                xps = xa.tile([128, 4, 272], bf16, tag="xps")
                sdh = xa.tile([128, 4, 8], bf16, tag="sdh")
                for ti in range(4):
                    ps = psa.tile([128, 264], f32, tag="psA", name="psA",
                                  bufs=2)
                    for kk in range(4):
                        sl = slice(ti * 128, (ti + 1) * 128)
                        nc.tensor.matmul(ps[:], lx[:, kk, sl], w1e_t[kk][:],
                                         start=(kk == 0), stop=(kk == 3))
                    nc.scalar.copy(xps[:, ti, 0:256], ps[:, 0:256])
                    # ss as f32 in slots 256:264
                    nc.vector.tensor_copy(
                        xps[:, ti, 256:272].bitcast(f32), ps[:, 256:264])
                    # sd hi/lo bf16 pairs for the logit AllGather
                    nc.vector.tensor_copy(sdh[:, ti, 0:4], ps[:, 260:264])
                    nc.vector.tensor_tensor(
                        sdh[:, ti, 4:8], ps[:, 260:264], sdh[:, ti, 0:4],
                        op=Alu.subtract)
                nc.sync.dma_start(
                    XPT[:].rearrange("(g4 p) c -> p g4 c", p=128)
                    [:, 4 * g:4 * g + 4, 0:264], xps[:, :, 0:264])
                if g < SEC // 512:
                    nc.sync.dma_start(
                        sd1in[:].rearrange("(g4 p) c -> p g4 c", p=128)
                        [:, 4 * g:4 * g + 4, :], sdh[:])

            for g in range(SEC // 512):
                phase_a_group(g)
            srcg1_t = cload([128, SLOTS1 // 16], i16, "srcg1", srcg1[:])
            dstl1_t = cload([128, SLOTS1 // 16], i16, "dstl1", dstl1[:])
            dstoffT1_t = cload([128, TILES1], f32, "dstoffT1", dstoffT1[:])
            wT1_t = cload([128, TILES1], f32, "wT1", wT1[:])
            iota_t = cload([128, 128], bf16, "iota", iota[:])
            wmue_t = [cload([128, 130], bf16, f"wmue{kk}",
                            wmue[kk * 128:(kk + 1) * 128, :])
                      for kk in range(2)]
            wlve_t = [cload([128, 130], bf16, f"wlve{kk}",
                            wlve[kk * 128:(kk + 1) * 128, :])
                      for kk in range(2)]
            b1b_t = cload([128, 256], f32, "b1b", b1b[:])
            bmub_t = cload([128, 128], f32, "bmub", bmub[:])
            blvb_t = cload([128, 128], f32, "blvb", blvb[:])
            ident_t = cload([128, 128], f32, "ident", ident[:])
            nc.gpsimd.collective_compute(
                "AllGather", mybir.AluOpType.bypass,
                replica_groups=[list(range(NC))],
                ins=[sd1in[:]], outs=[SD1T[:]])
            for s16 in range(16):
                nc.scalar.dma_start(
                    SDW1[:].rearrange("(r s) c -> r s c", s=16)
                    [:, s16, 0:8],
                    SD1T[:, s16 * 8:(s16 + 1) * 8])
            for g in range(SEC // 512, AUG // 512):
                phase_a_group(g)

            def edge_phase(layer, SRC_TBL, SDT_TBL, srcg_t, dofT, dstl_t,
                           wTt, TBLh, nchunk, tpb, rs_cb):
                nh = 4 if layer == 1 else 2
                blk_ps = {}
                ext = None
                qe = [min((QSTART[q + 1] * tpb + TPC - 1) // TPC + 5,
                          nchunk - 1) for q in range(3)]
                for ci in range(nchunk):
                    for q in range(3):
                        if ci == qe[q]:
                            rs_cb(q)
                    ohx_t = {}
                    for tt in range(TPC):
                        t = ci * TPC + tt
                        ohx = ohp.tile([128, 128], bf16, tag="ohx")
                        eng = nc.vector if tt % 4 != 3 else nc.gpsimd
                        eng.tensor_scalar(
                            ohx[:], iota_t[:], dofT[:, t:t + 1], None,
                            Alu.is_equal)
                        ohx_t[tt] = ohx
                    xrow = gx.tile([128, TPC, XW], bf16, tag="xrow")
                    HT = TPC // 2
                    HIC = IC // 2
                    for gh in range(2):
                        nc.gpsimd.dma_gather(
                            xrow[:, gh * HT:(gh + 1) * HT, :], SRC_TBL[:],
                            srcg_t[:, ci * IC + gh * HIC:
                                   ci * IC + (gh + 1) * HIC],
                            CHUNK // 2, CHUNK // 2, XW)
                    ext = ge.tile([128, TPC, 128], bf16, tag="ext")
                    for gh in range(2):
                        nc.gpsimd.dma_gather(
                            ext[:, gh * HT:(gh + 1) * HT, :], SDT_TBL[:],
                            dstl_t[:, ci * IC + gh * HIC:
                                   ci * IC + (gh + 1) * HIC],
                            CHUNK // 2, CHUNK // 2, 128)
                    exs = ext[:, :, 0:8]
                    z = sm.tile([128, TPC, nh], f32, tag="z")
                    if layer == 1:
                        nc.vector.tensor_tensor(
                            z[:], xrow[:, :, 256:264].bitcast(f32),
                            exs[:, :, 0:4], op=Alu.add)
                        nc.vector.tensor_tensor(
                            z[:], z[:], exs[:, :, 4:8], op=Alu.add)
                    else:
                        nc.vector.tensor_tensor(
                            z[:],
                            xrow[:, :, 256:264].bitcast(f32)[:, :, 0:2],
                            exs[:, :, 0:2], op=Alu.add)
                        nc.vector.tensor_tensor(
                            z[:], z[:], exs[:, :, 4:6], op=Alu.add)
                    nc.vector.scalar_tensor_tensor(
                        z[:], in0=z[:], scalar=NEG, in1=z[:],
                        op0=Alu.mult, op1=Alu.max)
                    ex = sm.tile([128, TPC, nh], f32, tag="ex")
                    nc.scalar.activation(ex[:], z[:], Act.Exp)
                    exw = sm.tile([128, TPC, nh], f32, tag="exw")
                    wb = wTt[:, ci * TPC:(ci + 1) * TPC]
                    nc.vector.tensor_tensor(
                        exw[:], ex[:],
                        wb.rearrange("p (t o) -> p t o", o=1).to_broadcast(
                            [128, TPC, nh]), op=Alu.mult)
                    exw2 = sm.tile([128, TPC, nh, 2], bf16, tag="exw2")
                    nc.vector.tensor_copy(
                        exw2[:], exw[:].rearrange("p t (h o) -> p t h o", o=1)
                        .to_broadcast([128, TPC, nh, 2]))
                    kw = 256 // nh // 2
                    xrh = xrow[:, :, 0:256].rearrange(
                        "p t (h k two) -> p t h k two", h=nh, two=2)
                    nc.vector.tensor_tensor(
                        xrh, xrh,
                        exw2[:].rearrange("p t h (o two) -> p t h o two",
                                          two=2)
                        .to_broadcast([128, TPC, nh, kw, 2]), op=Alu.mult)
                    nc.vector.tensor_copy(xrow[:, :, 256:256 + nh], ex[:])
                    nc.vector.tensor_tensor(
                        xrow[:, :, 256 + nh:256 + 2 * nh], ex[:],
                        xrow[:, :, 256:256 + nh], op=Alu.subtract)

                    for tt in range(TPC):
                        t = ci * TPC + tt
                        cell = t // tpb
                        k = t % tpb
                        if k == 0:
                            blk_ps[cell] = ps2.tile([128, 264], f32,
                                                    tag="blk", name="blkps",
                                                    bufs=4)
                        ps = blk_ps[cell]
                        nc.tensor.matmul(
                            ps[:, 0:264], ohx_t[tt][:], xrow[:, tt, 0:264],
                            start=(k == 0), stop=(k == tpb - 1))
                        if k == tpb - 1:
                            hf = max(q for q in range(NQ)
                                     if QSTART[q] <= cell)
                            rb = cell - QSTART[hf]
                            cpy = fin.tile([128, 264], bf16, tag="cpy",
                                           bufs=8)
                            nc.scalar.copy(cpy[:], ps[:, 0:264])
                            nc.sync.dma_start(
                                TBLh[hf][rb * 128:(rb + 1) * 128, :], cpy[:])
                            del blk_ps[cell]

            def emit_rs(TBLh, RSOh, hf):
                nc.gpsimd.collective_compute(
                    "ReduceScatter", Alu.add,
                    replica_groups=[list(range(NC))],
                    ins=[TBLh[hf][:]], outs=[RSOh[hf][:]])

            srcg2_t = cload([128, SLOTS2 // 16], i16, "srcg2", srcg2[:])
            dstoffT2_t = cload([128, TILES2], f32, "dstoffT2", dstoffT2[:])
            dstl2_t = cload([128, SLOTS2 // 16], i16, "dstl2", dstl2[:])
            wT2_t = cload([128, TILES2], f32, "wT2", wT2[:])
            # ---- L1 ----
            if upto >= 2:
                edge_phase(1, XPT, SDW1, srcg1_t, dstoffT1_t, dstl1_t, wT1_t,
                           TBL1, NCH1, TPB1,
                           (lambda hf: emit_rs(TBL1, RS1O, hf))
                           if upto >= 3 else (lambda hf: None))
            if upto >= 3:
                emit_rs(TBL1, RS1O, 3)

            # ---- L1 finalize ----
            for b in range(20 if upto >= 3 else 0):
                hf = max(q for q in range(NQ) if QLO[q] <= b)
                rbase = (b - QLO[hf]) * 128
                rsb = fin.tile([128, 264], bf16, tag="rsb")
                nc.sync.dma_start(rsb[:], RS1O[hf][rbase:rbase + 128, :])
                den = sm.tile([128, 4], f32, tag="den")
                nc.vector.tensor_tensor(den[:], rsb[:, 256:260],
                                        rsb[:, 260:264], op=Alu.add)
                nc.vector.tensor_scalar_add(den[:], den[:], EPS)
                rec = sm.tile([128, 4], f32, tag="rec")
                nc.vector.reciprocal(rec[:], den[:])
                hb = fin.tile([128, 256], f32, tag="hb")
                for h in range(H):
                    nc.vector.scalar_tensor_tensor(
                        hb[:, h * 64:(h + 1) * 64],
                        in0=rsb[:, h * 64:(h + 1) * 64],
                        scalar=rec[:, h:h + 1],
                        in1=b1b_t[:, h * 64:(h + 1) * 64],
                        op0=Alu.mult, op1=Alu.add)
                zm = fin.tile([128, 256], f32, tag="zm")
                nc.vector.tensor_scalar_min(zm[:], hb[:], 0.0)
                ez = fin.tile([128, 256], f32, tag="ez")
                nc.scalar.activation(ez[:], zm[:], Act.Exp)
                nc.vector.scalar_tensor_tensor(
                    hb[:], in0=hb[:], scalar=0.0, in1=ez[:],
                    op0=Alu.max, op1=Alu.add)
                nc.vector.tensor_scalar_add(hb[:], hb[:], -1.0)
                hTs = []
                for half in range(2):
                    pst = psa.tile([128, 264], f32, tag="psA", name="pstA",
                                   bufs=2)
                    nc.tensor.transpose(
                        pst[:, 0:128], hb[:, half * 128:(half + 1) * 128],
                        ident_t[:])
                    hT = fin.tile([128, 128], bf16, tag=f"hT{half}")
                    nc.vector.tensor_copy(hT[:], pst[:, 0:128])
                    hTs.append(hT)
                psmu_t = ps1.tile([128, 130], f32, tag="psmu", name="psmu")
                pslv_t = ps1.tile([128, 130], f32, tag="pslv", name="pslv")
                psmu = psmu_t[:]
                pslv = pslv_t[:]
                for kk in range(2):
                    nc.tensor.matmul(psmu, hTs[kk][:], wmue_t[kk][:],
                                     start=(kk == 0), stop=(kk == 1))
                    nc.tensor.matmul(pslv, hTs[kk][:], wlve_t[kk][:],
                                     start=(kk == 0), stop=(kk == 1))
                xr2 = fin.tile([128, 264], bf16, tag="xr2")
                nc.scalar.copy(xr2[:, 0:128], psmu[:, 0:128])
                nc.scalar.copy(xr2[:, 128:256], pslv[:, 0:128])
                # ss as f32 slots [ssmu, sslv]; sd hi/lo bf16 for AG2
                ssv = xr2[:, 256:264].bitcast(f32)
                nc.vector.tensor_copy(ssv[:, 0:1], psmu[:, 128:129])
                nc.vector.tensor_copy(ssv[:, 1:2], pslv[:, 128:129])
                sdh = fin.tile([128, 8], bf16, tag="sdh2")
                nc.vector.tensor_copy(sdh[:, 0:1], psmu[:, 129:130])
                nc.vector.tensor_copy(sdh[:, 1:2], pslv[:, 129:130])
                nc.vector.tensor_tensor(sdh[:, 4:5], psmu[:, 129:130],
                                        sdh[:, 0:1], op=Alu.subtract)
                nc.vector.tensor_tensor(sdh[:, 5:6], pslv[:, 129:130],
                                        sdh[:, 1:2], op=Alu.subtract)
                nc.sync.dma_start(MLT[b * 128:(b + 1) * 128, 0:264], xr2[:])
                nc.sync.dma_start(sd2in[b * 128:(b + 1) * 128, :], sdh[:])

            if upto >= 3:
                nc.gpsimd.collective_compute(
                    "AllGather", mybir.AluOpType.bypass,
                    replica_groups=[list(range(NC))],
                    ins=[sd2in[:]], outs=[SD2T[:]])
                for s16 in range(16):
                    nc.scalar.dma_start(
                        SDW2[:].rearrange("(r s) c -> r s c", s=16)
                        [:, s16, 0:8],
                        SD2T[:, s16 * 8:(s16 + 1) * 8])

            # ---- L2/3 ----
            if upto >= 4:
                edge_phase(2, MLT, SDW2, srcg2_t, dstoffT2_t, dstl2_t, wT2_t,
                           TBL2, NCH2, TPB2,
                           lambda hf: emit_rs(TBL2, RS2O, hf))
                emit_rs(TBL2, RS2O, 3)

            # ---- final ----
            for b in range(20 if upto >= 4 else 1):
                hf = max(q for q in range(NQ) if QLO[q] <= b)
                rbase = (b - QLO[hf]) * 128
                rsb = fin.tile([128, 264], bf16, tag="rsb2")
                nc.sync.dma_start(rsb[:], RS2O[hf][rbase:rbase + 128, :])
                for li, (bias_t, outdr) in enumerate(
                        ((bmub_t, mu_out), (blvb_t, lv_out))):
                    den = sm.tile([128, 1], f32, tag="den2")
                    nc.vector.tensor_tensor(
                        den[:], rsb[:, 256 + li:257 + li],
                        rsb[:, 258 + li:259 + li], op=Alu.add)
                    nc.vector.tensor_scalar_add(den[:], den[:], EPS)
                    rec = sm.tile([128, 1], f32, tag="rec2")
                    nc.vector.reciprocal(rec[:], den[:])
                    ob = fin.tile([128, 128], f32, tag="ob")
                    nc.vector.scalar_tensor_tensor(
                        ob[:], in0=rsb[:, li * 128:(li + 1) * 128],
                        scalar=rec[:, 0:1],
                        in1=bias_t[:], op0=Alu.mult, op1=Alu.add)
                    nc.sync.dma_start(
                        outdr[b * 128:(b + 1) * 128, :], ob[:])

    nc.compile()
    return nc


def _prep_inputs(x, edge_index, edge_weight, W1, att1, b1, Wmu, attmu, bmu,
                 Wlv, attlv, blv):
    import ml_dtypes
    bf = ml_dtypes.bfloat16

    src = np.asarray(edge_index[0], np.int64)
    dst = np.asarray(edge_index[1], np.int64)
    w = np.asarray(edge_weight, np.float32)
    x = np.asarray(x, np.float32)

    att1 = np.asarray(att1, np.float32)
    W1 = np.asarray(W1, np.float32)
    Wss1 = np.zeros((FIN, H), np.float32)
    Wsd1 = np.zeros((FIN, H), np.float32)
    for h in range(H):
        Wss1[:, h] = W1[:, h * C1:(h + 1) * C1] @ att1[h, C1:]
        Wsd1[:, h] = W1[:, h * C1:(h + 1) * C1] @ att1[h, :C1]
    w1e = np.concatenate([W1, Wss1, Wsd1], axis=1).astype(bf)

    attmu = np.asarray(attmu, np.float32).reshape(-1)
    attlv = np.asarray(attlv, np.float32).reshape(-1)
    Wmu = np.asarray(Wmu, np.float32)
    Wlv = np.asarray(Wlv, np.float32)
    wmue = np.concatenate(
        [Wmu, (Wmu @ attmu[LAT:])[:, None], (Wmu @ attmu[:LAT])[:, None]],
        axis=1).astype(bf)
    wlve = np.concatenate(
        [Wlv, (Wlv @ attlv[LAT:])[:, None], (Wlv @ attlv[:LAT])[:, None]],
        axis=1).astype(bf)

    xT_all = x.T.astype(bf)
    b1b = np.tile(np.asarray(b1, np.float32)[None, :], (128, 1))
    bmub = np.tile(np.asarray(bmu, np.float32)[None, :], (128, 1))
    blvb = np.tile(np.asarray(blv, np.float32)[None, :], (128, 1))
    iota = np.tile(np.arange(128, dtype=np.float32)[None, :],
                   (128, 1)).astype(bf)
    iotaP = np.ascontiguousarray(iota.T)
    ident = np.eye(128, dtype=np.float32)

    # ---- node permutation: LPT balance blocks by in-degree ----
    deg = np.bincount(dst, minlength=N).astype(np.int64)
    permrow = np.zeros(N, np.int64)     # node -> global padded row
    inv_rows = np.full(NC * SEC, -1, np.int64)  # row -> node
    for c in range(NC):
        nodes = np.arange(c * NOWN, (c + 1) * NOWN)
        order = nodes[np.argsort(-deg[nodes], kind="stable")]
        loads = np.zeros(20, np.int64)
        fill = np.zeros(20, np.int64)
        for nd in order:
            cand = np.where(fill < RPB)[0]
            bsel = cand[np.argmin(loads[cand])]
            permrow[nd] = c * SEC + bsel * 128 + fill[bsel]
            inv_rows[permrow[nd]] = nd
            loads[bsel] += deg[nd]
            fill[bsel] += 1

    prow = permrow[dst]
    gblk = prow // 128
    goff = prow % 128
    blktot = np.bincount(gblk, minlength=NBLK)
    assert blktot.max() <= NC * 256, f"block overflow {blktot.max()}"

    score = src // NOWN
    # L1 cell rebalancing: cap 256 per (core, block), move with halo
    assign = score.copy()
    eidx_by_cell = {}
    for b in range(NBLK):
        eb = np.where(gblk == b)[0]
        cores = assign[eb]
        cnt = np.bincount(cores, minlength=NC)
        over_c = [c for c in range(NC) if cnt[c] > 256]
        space = {c: 256 - int(cnt[c]) for c in range(NC)}
        movers = []
        for c in over_c:
            ec = eb[cores == c]
            movers.extend(ec[256:].tolist())
            space[c] = 0
        ptr = 0
        order = sorted(space, key=lambda k: -space[k])
        while ptr < len(movers):
            moved = False
            for c in order:
                if space[c] > 0 and ptr < len(movers):
                    assign[movers[ptr]] = c
                    space[c] -= 1
                    ptr += 1
                    moved = True
            assert moved
        assert ptr == len(movers)

    # local row maps: own nodes at their permuted slot, halo appended
    QSIZES = (7, 7, 5, 1)
    QLO = (0, 7, 14, 19)
    QSTART = (0, 56, 112, 152)
    lb = gblk % 20
    q_of = np.select([lb >= 19, lb >= 14, lb >= 7], [3, 2, 1], 0)
    qlo_a = np.array(QLO)[q_of]
    qsz_a = np.array(QSIZES)[q_of]
    qst_a = np.array(QSTART)[q_of]
    cell_of = qst_a + (gblk // 20) * qsz_a + (lb - qlo_a)

    in_maps = []
    for c in range(NC):
        m1 = assign == c
        e1 = np.where(m1)[0]
        own = score[e1] == c
        halo_nodes = np.unique(src[e1[~own]])
        assert SEC + len(halo_nodes) <= AUG, len(halo_nodes)
        lrow = np.full(N, -1, np.int64)
        sec_nodes = inv_rows[c * SEC:(c + 1) * SEC]
        valid = sec_nodes >= 0
        lrow[sec_nodes[valid]] = np.where(valid)[0]
        lrow[halo_nodes] = SEC + np.arange(len(halo_nodes))

        def pack(eids, tpb, tiles, slots):
            e_src = np.zeros(slots, np.int64)
            e_dst = np.zeros(slots, np.int64)
            e_off = np.full(slots, -1.0, np.float32)
            e_w = np.zeros(slots, np.float32)
            cells = cell_of[eids]
            for cell in range(160):
                bm = cells == cell
                ee = eids[bm]
                nbe = len(ee)
                assert nbe <= tpb * 128, (c, cell, nbe)
                o = cell * tpb * 128
                e_src[o:o + nbe] = lrow[src[ee]]
                e_dst[o:o + nbe] = prow[ee]
                e_off[o:o + nbe] = goff[ee].astype(np.float32)
                e_w[o:o + nbe] = w[ee]
            assert e_src.min() >= 0
            return e_src, e_dst, e_off, e_w

        s1, d1, o1, w1 = pack(e1, TPB1, TILES1, SLOTS1)
        e2 = np.where(score == c)[0]
        s2, d2, o2, w2 = pack(e2, TPB2, TILES2, SLOTS2)

        xTb_c = np.zeros((FIN, AUG), bf)
        cols = sec_nodes.copy()
        ok = cols >= 0
        xTb_c[:, :SEC][:, ok] = xT_all[:, cols[ok]]
        xTb_c[:, SEC:SEC + len(halo_nodes)] = xT_all[:, halo_nodes]

        in_maps.append({
            "xTb": xTb_c, "w1e": w1e, "wmue": wmue, "wlve": wlve, "b1b": b1b,
            "bmub": bmub, "blvb": blvb, "iota": iota, "iotaP": iotaP,
            "ident": ident,
            "srcg1": _wrap_idxs(s1), "srcg2": _wrap_idxs(s2),
            "dstoffT1": _colmajor(o1, TILES1),
            "dstoffT2": _colmajor(o2, TILES2),
            "dstl1": _wrap_idxs(d1), "dstl2": _wrap_idxs(d2),
            "wT1": _colmajor(w1, TILES1), "wT2": _colmajor(w2, TILES2),
        })
    return in_maps, inv_rows


def kernel(x, edge_index, edge_weight, W1, att1, b1, Wmu, attmu, bmu,
           Wlv, attlv, blv):
    from concourse.bass_utils import run_bass_kernel_spmd

    if "nc" not in _cache:
        _cache["nc"] = _build_module()
    nc = _cache["nc"]
    in_maps, inv_rows = _prep_inputs(x, edge_index, edge_weight, W1, att1, b1,
                                     Wmu, attmu, bmu, Wlv, attlv, blv)
    r = run_bass_kernel_spmd(nc, in_maps, list(range(NC)))
    mu = np.zeros((N, LAT), np.float32)
    lv = np.zeros((N, LAT), np.float32)
    for c in range(NC):
        rows = inv_rows[c * SEC:(c + 1) * SEC]
        ok = rows >= 0
        mu[rows[ok]] = r.results[c]["mu_out"][ok]
        lv[rows[ok]] = r.results[c]["lv_out"][ok]
    return (mu, lv)


# revision 13
# speedup vs baseline: 1.0380x; 1.0121x over previous
"""GAT encoder on 8 trn2 cores — src-sharded edges + ReduceScatter partials.

Strategy:
 - Nodes are permuted within each core's section (20 blocks x 125 real + 3
   pad rows), LPT-balanced by in-degree so every global dst block receives
   ~2000 edges (<= 2048).
 - Edges are processed by the core owning their SRC node. For layer 1,
   per-(core,block) overflow beyond 256 edges is moved to under-loaded cores
   with the src row replicated there (halo, ~500 rows/core), giving a
   uniform 2 tiles per (core,block) cell: 320 tiles, 40960 slots (2% pad).
   Layer 2 keeps src-owner assignment with 3 tiles/cell (no halo possible
   for projected features).
 - Phase A projects only own+halo nodes (1/8 of the baseline's replicated
   work); only the tiny per-node dst logits are AllGathered (16B/node,
   bf16 hi/lo pairs).
 - Per-edge dst logits come from a transposed one-hot matmul on the PE
   against an SBUF-resident logit table (no 256B/edge DMA gather).
 - Aggregation per dst block via one-hot matmuls into PSUM; partials
   (payload + denominator hi/lo) land in a [20480, 264] bf16 table split in
   two block-halves; a ReduceScatter(add) per half hands each core its own
   fully-reduced rows. The first RS overlaps the second half of each edge
   phase. Replaces the baseline's 330us serialized feature-AllGather chain.
Outputs (mu, logvar) assembled host-side (un-permuted) from per-core slices.
"""

import numpy as np

# ---- problem constants ----
N = 20000
E = 320000
FIN = 512
HID = 256
LAT = 128
H = 4
C1 = 64
NEG = 0.2
EPS = 1e-16

NC = 8
NOWN = 2500
SEC = 2560               # padded section rows (20 blocks)
NBLK = NC * 20           # 160 global dst blocks
RPB = 125                # real nodes per block
AUG = 3584               # local src table rows (2560 own + 1024 halo)
XW = 384                 # physical row width of gather tables (768B)
CW = 264                 # used row width / partial table width

TPB1 = 2                 # L1 tiles per cell
TILES1 = NBLK * TPB1     # 320
SLOTS1 = TILES1 * 128    # 40960
TPB2 = 3                 # L2 tiles per cell
TILES2 = NBLK * TPB2     # 480
SLOTS2 = TILES2 * 128    # 61440
TPC = 16                 # tiles per chunk
CHUNK = TPC * 128        # 2048
IC = CHUNK // 16         # idx cols per chunk
NCH1 = TILES1 // TPC     # 20
NCH2 = TILES2 // TPC     # 30
NQ = 4                   # ReduceScatter splits (uneven)
QSIZES = (7, 7, 5, 1)    # blocks per quarter; last tiny to shrink the tail
QLO = (0, 7, 14, 19)     # first block of each quarter
QSTART = (0, 56, 112, 152)  # first cell index of each quarter

_cache = {}


def _wrap_idxs(idx):
    n = idx.shape[0]
    t = np.zeros((128, n // 16), np.int16)
    w = idx.reshape(n // 16, 16).T.astype(np.int16)
    for g in range(8):
        t[g * 16:(g + 1) * 16, :] = w
    return t


def _colmajor(a, tiles):
    return np.ascontiguousarray(a.reshape(tiles, 128).T)


def _rowmajor_tiles(a, tiles):
    # per-tile rows for the transposed one-hot build: [128, ceil(T/128), 128]
    reps = (tiles + 127) // 128
    out = np.zeros((128, reps, 128), a.dtype)
    ar = a.reshape(tiles, 128)
    for t in range(tiles):
        out[t % 128, t // 128, :] = ar[t]
    return out


def _build_module(upto=4):
    import concourse.bacc as bacc
    import concourse.mybir as mybir
    import concourse.tile as tile

    f32 = mybir.dt.float32
    bf16 = mybir.dt.bfloat16
    i16 = mybir.dt.int16
    Alu = mybir.AluOpType
    Act = mybir.ActivationFunctionType

    nc = bacc.Bacc("TRN2", target_bir_lowering=False, num_devices=NC,
                   dynamic_dma_scratch_size=65536)

    # ---- inputs ----
    xTb = nc.dram_tensor("xTb", [FIN, AUG], bf16, kind="ExternalInput")
    w1e = nc.dram_tensor("w1e", [FIN, 264], bf16, kind="ExternalInput")
    wmue = nc.dram_tensor("wmue", [HID, 130], bf16, kind="ExternalInput")
    wlve = nc.dram_tensor("wlve", [HID, 130], bf16, kind="ExternalInput")
    b1b = nc.dram_tensor("b1b", [128, 256], f32, kind="ExternalInput")
    bmub = nc.dram_tensor("bmub", [128, 128], f32, kind="ExternalInput")
    blvb = nc.dram_tensor("blvb", [128, 128], f32, kind="ExternalInput")
    iota = nc.dram_tensor("iota", [128, 128], bf16, kind="ExternalInput")
    ident = nc.dram_tensor("ident", [128, 128], f32, kind="ExternalInput")
    srcg1 = nc.dram_tensor("srcg1", [128, SLOTS1 // 16], i16,
                           kind="ExternalInput")
    srcg2 = nc.dram_tensor("srcg2", [128, SLOTS2 // 16], i16,
                           kind="ExternalInput")
    dstoffT1 = nc.dram_tensor("dstoffT1", [128, TILES1], f32,
                              kind="ExternalInput")
    dstoffT2 = nc.dram_tensor("dstoffT2", [128, TILES2], f32,
                              kind="ExternalInput")
    dstl1 = nc.dram_tensor("dstl1", [128, SLOTS1 // 16], i16,
                           kind="ExternalInput")
    dstl2 = nc.dram_tensor("dstl2", [128, SLOTS2 // 16], i16,
                           kind="ExternalInput")
    wT1 = nc.dram_tensor("wT1", [128, TILES1], f32, kind="ExternalInput")
    wT2 = nc.dram_tensor("wT2", [128, TILES2], f32, kind="ExternalInput")

    mu_out = nc.dram_tensor("mu_out", [SEC, LAT], f32, kind="ExternalOutput")
    lv_out = nc.dram_tensor("lv_out", [SEC, LAT], f32, kind="ExternalOutput")

    with tile.TileContext(nc) as tc:
        with (
            tc.tile_pool(name="cst", bufs=1) as cst,
            tc.tile_pool(name="lw", bufs=2) as lw,
            tc.tile_pool(name="xa", bufs=3) as xa,
            tc.tile_pool(name="gx", bufs=3) as gx,
            tc.tile_pool(name="oh", bufs=44) as ohp,
            tc.tile_pool(name="sm", bufs=8) as sm,
            tc.tile_pool(name="fin", bufs=4) as fin,
            tc.tile_pool(name="ps2", bufs=3, space="PSUM") as ps2,
            tc.tile_pool(name="psa", bufs=1, space="PSUM") as psa,
            tc.tile_pool(name="ps1", bufs=1, space="PSUM") as ps1,
            tc.tile_pool(name="ge", bufs=4) as ge,
            tc.tile_pool(name="dr", bufs=1, space="DRAM") as dr,
        ):
            XPT = dr.tile([AUG, XW], bf16, tag="XPT")
            MLT = dr.tile([SEC, XW], bf16, tag="MLT")
            sd1in = dr.tile([SEC, 8], bf16, tag="sd1in")
            SD1T = dr.tile([NC * SEC // 16, 128], bf16, tag="SD1T",
                           addr_space="Shared")
            SDW1 = dr.tile([NC * SEC, 128], bf16, tag="SDW1")
            sd2in = dr.tile([SEC, 8], bf16, tag="sd2in")
            SD2T = dr.tile([NC * SEC // 16, 128], bf16, tag="SD2T",
                           addr_space="Shared")
            SDW2 = dr.tile([NC * SEC, 128], bf16, tag="SDW2")
            TBL1 = []
            TBL2 = []
            RS1O = []
            RS2O = []
            for hf in range(NQ):
                qr = QSIZES[hf] * 128
                TBL1.append(dr.tile([NC * qr, CW], bf16, tag=f"TBL1{hf}",
                                    name=f"TBL1{hf}"))
                TBL2.append(dr.tile([NC * qr, CW], bf16, tag=f"TBL2{hf}",
                                    name=f"TBL2{hf}"))
                RS1O.append(dr.tile([qr, CW], bf16, tag=f"RS1O{hf}",
                                    name=f"RS1O{hf}"))
                RS2O.append(dr.tile([qr, CW], bf16, tag=f"RS2O{hf}",
                                    name=f"RS2O{hf}"))

            # resident constants
            def cload(shape, dtype, tag, srcap):
                t = cst.tile(shape, dtype, tag=tag)
                nc.sync.dma_start(t[:], srcap)
                return t

            w1e_t = [cload([128, 264], bf16, f"w1e{kk}",
                           w1e[kk * 128:(kk + 1) * 128, :]) for kk in range(4)]

            # ---- phase A: own groups, then AG1, then halo group ----
            def phase_a_group(g):
                lx = lw.tile([128, 4, 512], bf16, tag="lx")
                nc.sync.dma_start(
                    lx[:], xTb[:].rearrange("(kk p) (g n) -> p kk g n",
                                            p=128, n=512)[:, :, g, :])
                xps = xa.tile([128, 4, 272], bf16, tag="xps")
                sdh = xa.tile([128, 4, 8], bf16, tag="sdh")
                for ti in range(4):
                    ps = psa.tile([128, 264], f32, tag="psA", name="psA",
                                  bufs=2)
                    for kk in range(4):
                        sl = slice(ti * 128, (ti + 1) * 128)
                        nc.tensor.matmul(ps[:], lx[:, kk, sl], w1e_t[kk][:],
                                         start=(kk == 0), stop=(kk == 3))
                    nc.scalar.copy(xps[:, ti, 0:256], ps[:, 0:256])
                    # ss as f32 in slots 256:264
                    nc.vector.tensor_copy(
                        xps[:, ti, 256:272].bitcast(f32), ps[:, 256:264])
                    # sd hi/lo bf16 pairs for the logit AllGather
                    nc.vector.tensor_copy(sdh[:, ti, 0:4], ps[:, 260:264])
                    nc.vector.tensor_tensor(
                        sdh[:, ti, 4:8], ps[:, 260:264], sdh[:, ti, 0:4],
                        op=Alu.subtract)
                nc.sync.dma_start(
                    XPT[:].rearrange("(g4 p) c -> p g4 c", p=128)
                    [:, 4 * g:4 * g + 4, 0:264], xps[:, :, 0:264])
                if g < SEC // 512:
                    nc.sync.dma_start(
                        sd1in[:].rearrange("(g4 p) c -> p g4 c", p=128)
                        [:, 4 * g:4 * g + 4, :], sdh[:])

            for g in range(SEC // 512):
                phase_a_group(g)
            srcg1_t = cload([128, SLOTS1 // 16], i16, "srcg1", srcg1[:])
            dstl1_t = cload([128, SLOTS1 // 16], i16, "dstl1", dstl1[:])
            dstoffT1_t = cload([128, TILES1], f32, "dstoffT1", dstoffT1[:])
            wT1_t = cload([128, TILES1], f32, "wT1", wT1[:])
            iota_t = cload([128, 128], bf16, "iota", iota[:])
            wmue_t = [cload([128, 130], bf16, f"wmue{kk}",
                            wmue[kk * 128:(kk + 1) * 128, :])
                      for kk in range(2)]
            wlve_t = [cload([128, 130], bf16, f"wlve{kk}",
                            wlve[kk * 128:(kk + 1) * 128, :])
                      for kk in range(2)]
            b1b_t = cload([128, 256], f32, "b1b", b1b[:])
            bmub_t = cload([128, 128], f32, "bmub", bmub[:])
            blvb_t = cload([128, 128], f32, "blvb", blvb[:])
            ident_t = cload([128, 128], f32, "ident", ident[:])
            nc.gpsimd.collective_compute(
                "AllGather", mybir.AluOpType.bypass,
                replica_groups=[list(range(NC))],
                ins=[sd1in[:]], outs=[SD1T[:]])
            for s16 in range(16):
                nc.scalar.dma_start(
                    SDW1[:].rearrange("(r s) c -> r s c", s=16)
                    [:, s16, 0:8],
                    SD1T[:, s16 * 8:(s16 + 1) * 8])
            for g in range(SEC // 512, AUG // 512):
                phase_a_group(g)

            def edge_phase(layer, SRC_TBL, SDT_TBL, srcg_t, dofT, dstl_t,
                           wTt, TBLh, nchunk, tpb, rs_cb):
                nh = 4 if layer == 1 else 2
                blk_ps = {}
                ext = None
                qe = [min((QSTART[q + 1] * tpb + TPC - 1) // TPC + 4,
                          nchunk - 1) for q in range(3)]
                for ci in range(nchunk):
                    for q in range(3):
                        if ci == qe[q]:
                            rs_cb(q)
                    ohx_t = {}
                    for tt in range(TPC):
                        t = ci * TPC + tt
                        ohx = ohp.tile([128, 128], bf16, tag="ohx")
                        eng = nc.vector if tt % 4 != 3 else nc.gpsimd
                        eng.tensor_scalar(
                            ohx[:], iota_t[:], dofT[:, t:t + 1], None,
                            Alu.is_equal)
                        ohx_t[tt] = ohx
                    xrow = gx.tile([128, TPC, XW], bf16, tag="xrow")
                    HT = TPC // 2
                    HIC = IC // 2
                    for gh in range(2):
                        nc.gpsimd.dma_gather(
                            xrow[:, gh * HT:(gh + 1) * HT, :], SRC_TBL[:],
                            srcg_t[:, ci * IC + gh * HIC:
                                   ci * IC + (gh + 1) * HIC],
                            CHUNK // 2, CHUNK // 2, XW)
                    ext = ge.tile([128, TPC, 128], bf16, tag="ext")
                    for gh in range(2):
                        nc.gpsimd.dma_gather(
                            ext[:, gh * HT:(gh + 1) * HT, :], SDT_TBL[:],
                            dstl_t[:, ci * IC + gh * HIC:
                                   ci * IC + (gh + 1) * HIC],
                            CHUNK // 2, CHUNK // 2, 128)
                    exs = ext[:, :, 0:8]
                    z = sm.tile([128, TPC, nh], f32, tag="z")
                    if layer == 1:
                        nc.vector.tensor_tensor(
                            z[:], xrow[:, :, 256:264].bitcast(f32),
                            exs[:, :, 0:4], op=Alu.add)
                        nc.vector.tensor_tensor(
                            z[:], z[:], exs[:, :, 4:8], op=Alu.add)
                    else:
                        nc.vector.tensor_tensor(
                            z[:],
                            xrow[:, :, 256:264].bitcast(f32)[:, :, 0:2],
                            exs[:, :, 0:2], op=Alu.add)
                        nc.vector.tensor_tensor(
                            z[:], z[:], exs[:, :, 4:6], op=Alu.add)
                    nc.vector.scalar_tensor_tensor(
                        z[:], in0=z[:], scalar=NEG, in1=z[:],
                        op0=Alu.mult, op1=Alu.max)
                    ex = sm.tile([128, TPC, nh], f32, tag="ex")
                    nc.scalar.activation(ex[:], z[:], Act.Exp)
                    exw = sm.tile([128, TPC, nh], f32, tag="exw")
                    wb = wTt[:, ci * TPC:(ci + 1) * TPC]
                    nc.vector.tensor_tensor(
                        exw[:], ex[:],
                        wb.rearrange("p (t o) -> p t o", o=1).to_broadcast(
                            [128, TPC, nh]), op=Alu.mult)
                    exw2 = sm.tile([128, TPC, nh, 2], bf16, tag="exw2")
                    nc.vector.tensor_copy(
                        exw2[:], exw[:].rearrange("p t (h o) -> p t h o", o=1)
                        .to_broadcast([128, TPC, nh, 2]))
                    kw = 256 // nh // 2
                    xrh = xrow[:, :, 0:256].rearrange(
                        "p t (h k two) -> p t h k two", h=nh, two=2)
                    nc.vector.tensor_tensor(
                        xrh, xrh,
                        exw2[:].rearrange("p t h (o two) -> p t h o two",
                                          two=2)
                        .to_broadcast([128, TPC, nh, kw, 2]), op=Alu.mult)
                    nc.vector.tensor_copy(xrow[:, :, 256:256 + nh], ex[:])
                    nc.vector.tensor_tensor(
                        xrow[:, :, 256 + nh:256 + 2 * nh], ex[:],
                        xrow[:, :, 256:256 + nh], op=Alu.subtract)

                    for tt in range(TPC):
                        t = ci * TPC + tt
                        cell = t // tpb
                        k = t % tpb
                        if k == 0:
                            blk_ps[cell] = ps2.tile([128, 264], f32,
                                                    tag="blk", name="blkps",
                                                    bufs=4)
                        ps = blk_ps[cell]
                        nc.tensor.matmul(
                            ps[:, 0:264], ohx_t[tt][:], xrow[:, tt, 0:264],
                            start=(k == 0), stop=(k == tpb - 1))
                        if k == tpb - 1:
                            hf = max(q for q in range(NQ)
                                     if QSTART[q] <= cell)
                            rb = cell - QSTART[hf]
                            cpy = fin.tile([128, 264], bf16, tag="cpy",
                                           bufs=8)
                            nc.scalar.copy(cpy[:], ps[:, 0:264])
                            nc.sync.dma_start(
                                TBLh[hf][rb * 128:(rb + 1) * 128, :], cpy[:])
                            del blk_ps[cell]

            def emit_rs(TBLh, RSOh, hf):
                nc.gpsimd.collective_compute(
                    "ReduceScatter", Alu.add,
                    replica_groups=[list(range(NC))],
                    ins=[TBLh[hf][:]], outs=[RSOh[hf][:]])

            srcg2_t = cload([128, SLOTS2 // 16], i16, "srcg2", srcg2[:])
            dstoffT2_t = cload([128, TILES2], f32, "dstoffT2", dstoffT2[:])
            dstl2_t = cload([128, SLOTS2 // 16], i16, "dstl2", dstl2[:])
            wT2_t = cload([128, TILES2], f32, "wT2", wT2[:])
            # ---- L1 ----
            if upto >= 2:
                edge_phase(1, XPT, SDW1, srcg1_t, dstoffT1_t, dstl1_t, wT1_t,
                           TBL1, NCH1, TPB1,
                           (lambda hf: emit_rs(TBL1, RS1O, hf))
                           if upto >= 3 else (lambda hf: None))
            if upto >= 3:
                emit_rs(TBL1, RS1O, 3)

            # ---- L1 finalize ----
            for b in range(20 if upto >= 3 else 0):
                hf = max(q for q in range(NQ) if QLO[q] <= b)
                rbase = (b - QLO[hf]) * 128
                rsb = fin.tile([128, 264], bf16, tag="rsb")
                nc.sync.dma_start(rsb[:], RS1O[hf][rbase:rbase + 128, :])
                den = sm.tile([128, 4], f32, tag="den")
                nc.vector.tensor_tensor(den[:], rsb[:, 256:260],
                                        rsb[:, 260:264], op=Alu.add)
                nc.vector.tensor_scalar_add(den[:], den[:], EPS)
                rec = sm.tile([128, 4], f32, tag="rec")
                nc.vector.reciprocal(rec[:], den[:])
                hb = fin.tile([128, 256], f32, tag="hb")
                for h in range(H):
                    nc.vector.scalar_tensor_tensor(
                        hb[:, h * 64:(h + 1) * 64],
                        in0=rsb[:, h * 64:(h + 1) * 64],
                        scalar=rec[:, h:h + 1],
                        in1=b1b_t[:, h * 64:(h + 1) * 64],
                        op0=Alu.mult, op1=Alu.add)
                zm = fin.tile([128, 256], f32, tag="zm")
                nc.vector.tensor_scalar_min(zm[:], hb[:], 0.0)
                ez = fin.tile([128, 256], f32, tag="ez")
                nc.scalar.activation(ez[:], zm[:], Act.Exp)
                nc.vector.scalar_tensor_tensor(
                    hb[:], in0=hb[:], scalar=0.0, in1=ez[:],
                    op0=Alu.max, op1=Alu.add)
                nc.vector.tensor_scalar_add(hb[:], hb[:], -1.0)
                hTs = []
                for half in range(2):
                    pst = psa.tile([128, 264], f32, tag="psA", name="pstA",
                                   bufs=2)
                    nc.tensor.transpose(
                        pst[:, 0:128], hb[:, half * 128:(half + 1) * 128],
                        ident_t[:])
                    hT = fin.tile([128, 128], bf16, tag=f"hT{half}")
                    nc.vector.tensor_copy(hT[:], pst[:, 0:128])
                    hTs.append(hT)
                psmu_t = ps1.tile([128, 130], f32, tag="psmu", name="psmu")
                pslv_t = ps1.tile([128, 130], f32, tag="pslv", name="pslv")
                psmu = psmu_t[:]
                pslv = pslv_t[:]
                for kk in range(2):
                    nc.tensor.matmul(psmu, hTs[kk][:], wmue_t[kk][:],
                                     start=(kk == 0), stop=(kk == 1))
                    nc.tensor.matmul(pslv, hTs[kk][:], wlve_t[kk][:],
                                     start=(kk == 0), stop=(kk == 1))
                xr2 = fin.tile([128, 264], bf16, tag="xr2")
                nc.scalar.copy(xr2[:, 0:128], psmu[:, 0:128])
                nc.scalar.copy(xr2[:, 128:256], pslv[:, 0:128])
                # ss as f32 slots [ssmu, sslv]; sd hi/lo bf16 for AG2
                ssv = xr2[:, 256:264].bitcast(f32)
                nc.vector.tensor_copy(ssv[:, 0:1], psmu[:, 128:129])
                nc.vector.tensor_copy(ssv[:, 1:2], pslv[:, 128:129])
                sdh = fin.tile([128, 8], bf16, tag="sdh2")
                nc.vector.tensor_copy(sdh[:, 0:1], psmu[:, 129:130])
                nc.vector.tensor_copy(sdh[:, 1:2], pslv[:, 129:130])
                nc.vector.tensor_tensor(sdh[:, 4:5], psmu[:, 129:130],
                                        sdh[:, 0:1], op=Alu.subtract)
                nc.vector.tensor_tensor(sdh[:, 5:6], pslv[:, 129:130],
                                        sdh[:, 1:2], op=Alu.subtract)
                nc.sync.dma_start(MLT[b * 128:(b + 1) * 128, 0:264], xr2[:])
                nc.sync.dma_start(sd2in[b * 128:(b + 1) * 128, :], sdh[:])

            if upto >= 3:
                nc.gpsimd.collective_compute(
                    "AllGather", mybir.AluOpType.bypass,
                    replica_groups=[list(range(NC))],
                    ins=[sd2in[:]], outs=[SD2T[:]])
                for s16 in range(16):
                    nc.scalar.dma_start(
                        SDW2[:].rearrange("(r s) c -> r s c", s=16)
                        [:, s16, 0:8],
                        SD2T[:, s16 * 8:(s16 + 1) * 8])

            # ---- L2/3 ----
            if upto >= 4:
                edge_phase(2, MLT, SDW2, srcg2_t, dstoffT2_t, dstl2_t, wT2_t,
                           TBL2, NCH2, TPB2,
                           lambda hf: emit_rs(TBL2, RS2O, hf))
                emit_rs(TBL2, RS2O, 3)

            # ---- final ----
            for b in range(20 if upto >= 4 else 1):
                hf = max(q for q in range(NQ) if QLO[q] <= b)
                rbase = (b - QLO[hf]) * 128
                rsb = fin.tile([128, 264], bf16, tag="rsb2")
                nc.sync.dma_start(rsb[:], RS2O[hf][rbase:rbase + 128, :])
                for li, (bias_t, outdr) in enumerate(
                        ((bmub_t, mu_out), (blvb_t, lv_out))):
                    den = sm.tile([128, 1], f32, tag="den2")
                    nc.vector.tensor_tensor(
                        den[:], rsb[:, 256 + li:257 + li],
                        rsb[:, 258 + li:259 + li], op=Alu.add)
                    nc.vector.tensor_scalar_add(den[:], den[:], EPS)
                    rec = sm.tile([128, 1], f32, tag="rec2")
                    nc.vector.reciprocal(rec[:], den[:])
                    ob = fin.tile([128, 128], f32, tag="ob")
                    nc.vector.scalar_tensor_tensor(
                        ob[:], in0=rsb[:, li * 128:(li + 1) * 128],
                        scalar=rec[:, 0:1],
                        in1=bias_t[:], op0=Alu.mult, op1=Alu.add)
                    nc.sync.dma_start(
                        outdr[b * 128:(b + 1) * 128, :], ob[:])

    nc.compile()
    return nc


def _prep_inputs(x, edge_index, edge_weight, W1, att1, b1, Wmu, attmu, bmu,
                 Wlv, attlv, blv):
    import ml_dtypes
    bf = ml_dtypes.bfloat16

    src = np.asarray(edge_index[0], np.int64)
    dst = np.asarray(edge_index[1], np.int64)
    w = np.asarray(edge_weight, np.float32)
    x = np.asarray(x, np.float32)

    att1 = np.asarray(att1, np.float32)
    W1 = np.asarray(W1, np.float32)
    Wss1 = np.zeros((FIN, H), np.float32)
    Wsd1 = np.zeros((FIN, H), np.float32)
    for h in range(H):
        Wss1[:, h] = W1[:, h * C1:(h + 1) * C1] @ att1[h, C1:]
        Wsd1[:, h] = W1[:, h * C1:(h + 1) * C1] @ att1[h, :C1]
    w1e = np.concatenate([W1, Wss1, Wsd1], axis=1).astype(bf)

    attmu = np.asarray(attmu, np.float32).reshape(-1)
    attlv = np.asarray(attlv, np.float32).reshape(-1)
    Wmu = np.asarray(Wmu, np.float32)
    Wlv = np.asarray(Wlv, np.float32)
    wmue = np.concatenate(
        [Wmu, (Wmu @ attmu[LAT:])[:, None], (Wmu @ attmu[:LAT])[:, None]],
        axis=1).astype(bf)
    wlve = np.concatenate(
        [Wlv, (Wlv @ attlv[LAT:])[:, None], (Wlv @ attlv[:LAT])[:, None]],
        axis=1).astype(bf)

    xT_all = x.T.astype(bf)
    b1b = np.tile(np.asarray(b1, np.float32)[None, :], (128, 1))
    bmub = np.tile(np.asarray(bmu, np.float32)[None, :], (128, 1))
    blvb = np.tile(np.asarray(blv, np.float32)[None, :], (128, 1))
    iota = np.tile(np.arange(128, dtype=np.float32)[None, :],
                   (128, 1)).astype(bf)
    iotaP = np.ascontiguousarray(iota.T)
    ident = np.eye(128, dtype=np.float32)

    # ---- node permutation: LPT balance blocks by in-degree ----
    deg = np.bincount(dst, minlength=N).astype(np.int64)
    permrow = np.zeros(N, np.int64)     # node -> global padded row
    inv_rows = np.full(NC * SEC, -1, np.int64)  # row -> node
    for c in range(NC):
        nodes = np.arange(c * NOWN, (c + 1) * NOWN)
        order = nodes[np.argsort(-deg[nodes], kind="stable")]
        loads = np.zeros(20, np.int64)
        fill = np.zeros(20, np.int64)
        for nd in order:
            cand = np.where(fill < RPB)[0]
            bsel = cand[np.argmin(loads[cand])]
            permrow[nd] = c * SEC + bsel * 128 + fill[bsel]
            inv_rows[permrow[nd]] = nd
            loads[bsel] += deg[nd]
            fill[bsel] += 1

    prow = permrow[dst]
    gblk = prow // 128
    goff = prow % 128
    blktot = np.bincount(gblk, minlength=NBLK)
    assert blktot.max() <= NC * 256, f"block overflow {blktot.max()}"

    score = src // NOWN
    # L1 cell rebalancing: cap 256 per (core, block), move with halo
    assign = score.copy()
    eidx_by_cell = {}
    for b in range(NBLK):
        eb = np.where(gblk == b)[0]
        cores = assign[eb]
        cnt = np.bincount(cores, minlength=NC)
        over_c = [c for c in range(NC) if cnt[c] > 256]
        space = {c: 256 - int(cnt[c]) for c in range(NC)}
        movers = []
        for c in over_c:
            ec = eb[cores == c]
            movers.extend(ec[256:].tolist())
            space[c] = 0
        ptr = 0
        order = sorted(space, key=lambda k: -space[k])
        while ptr < len(movers):
            moved = False
            for c in order:
                if space[c] > 0 and ptr < len(movers):
                    assign[movers[ptr]] = c
                    space[c] -= 1
                    ptr += 1
                    moved = True
            assert moved
        assert ptr == len(movers)

    # local row maps: own nodes at their permuted slot, halo appended
    QSIZES = (7, 7, 5, 1)
    QLO = (0, 7, 14, 19)
    QSTART = (0, 56, 112, 152)
    lb = gblk % 20
    q_of = np.select([lb >= 19, lb >= 14, lb >= 7], [3, 2, 1], 0)
    qlo_a = np.array(QLO)[q_of]
    qsz_a = np.array(QSIZES)[q_of]
    qst_a = np.array(QSTART)[q_of]
    cell_of = qst_a + (gblk // 20) * qsz_a + (lb - qlo_a)

    in_maps = []
    for c in range(NC):
        m1 = assign == c
        e1 = np.where(m1)[0]
        own = score[e1] == c
        halo_nodes = np.unique(src[e1[~own]])
        assert SEC + len(halo_nodes) <= AUG, len(halo_nodes)
        lrow = np.full(N, -1, np.int64)
        sec_nodes = inv_rows[c * SEC:(c + 1) * SEC]
        valid = sec_nodes >= 0
        lrow[sec_nodes[valid]] = np.where(valid)[0]
        lrow[halo_nodes] = SEC + np.arange(len(halo_nodes))

        def pack(eids, tpb, tiles, slots):
            e_src = np.zeros(slots, np.int64)
            e_dst = np.zeros(slots, np.int64)
            e_off = np.full(slots, -1.0, np.float32)
            e_w = np.zeros(slots, np.float32)
            cells = cell_of[eids]
            for cell in range(160):
                bm = cells == cell
                ee = eids[bm]
                nbe = len(ee)
                assert nbe <= tpb * 128, (c, cell, nbe)
                o = cell * tpb * 128
                e_src[o:o + nbe] = lrow[src[ee]]
                e_dst[o:o + nbe] = prow[ee]
                e_off[o:o + nbe] = goff[ee].astype(np.float32)
                e_w[o:o + nbe] = w[ee]
            assert e_src.min() >= 0
            return e_src, e_dst, e_off, e_w

        s1, d1, o1, w1 = pack(e1, TPB1, TILES1, SLOTS1)
        e2 = np.where(score == c)[0]
        s2, d2, o2, w2 = pack(e2, TPB2, TILES2, SLOTS2)

        xTb_c = np.zeros((FIN, AUG), bf)
        cols = sec_nodes.copy()
        ok = cols >= 0
        xTb_c[:, :SEC][:, ok] = xT_all[:, cols[ok]]
        xTb_c[:, SEC:SEC + len(halo_nodes)] = xT_all[:, halo_nodes]

        in_maps.append({
            "xTb": xTb_c, "w1e": w1e, "wmue": wmue, "wlve": wlve, "b1b": b1b,
            "bmub": bmub, "blvb": blvb, "iota": iota, "iotaP": iotaP,
            "ident": ident,
            "srcg1": _wrap_idxs(s1), "srcg2": _wrap_idxs(s2),
            "dstoffT1": _colmajor(o1, TILES1),
            "dstoffT2": _colmajor(o2, TILES2),
            "dstl1": _wrap_idxs(d1), "dstl2": _wrap_idxs(d2),
            "wT1": _colmajor(w1, TILES1), "wT2": _colmajor(w2, TILES2),
        })
    return in_maps, inv_rows


def kernel(x, edge_index, edge_weight, W1, att1, b1, Wmu, attmu, bmu,
           Wlv, attlv, blv):
    from concourse.bass_utils import run_bass_kernel_spmd

    if "nc" not in _cache:
        _cache["nc"] = _build_module()
    nc = _cache["nc"]
    in_maps, inv_rows = _prep_inputs(x, edge_index, edge_weight, W1, att1, b1,
                                     Wmu, attmu, bmu, Wlv, attlv, blv)
    r = run_bass_kernel_spmd(nc, in_maps, list(range(NC)))
    mu = np.zeros((N, LAT), np.float32)
    lv = np.zeros((N, LAT), np.float32)
    for c in range(NC):
        rows = inv_rows[c * SEC:(c + 1) * SEC]
        ok = rows >= 0
        mu[rows[ok]] = r.results[c]["mu_out"][ok]
        lv[rows[ok]] = r.results[c]["lv_out"][ok]
    return (mu, lv)


# revision 35
# speedup vs baseline: 1.0490x; 1.0105x over previous
"""GAT encoder on 8 trn2 cores — src-sharded edges + ReduceScatter partials.

Strategy:
 - Nodes are permuted within each core's section (20 blocks x 125 real + 3
   pad rows), LPT-balanced by in-degree so every global dst block receives
   ~2000 edges (<= 2048).
 - Edges are processed by the core owning their SRC node. For layer 1,
   per-(core,block) overflow beyond 256 edges is moved to under-loaded cores
   with the src row replicated there (halo, ~500 rows/core), giving a
   uniform 2 tiles per (core,block) cell: 320 tiles, 40960 slots (2% pad).
   Layer 2 keeps src-owner assignment with 3 tiles/cell (no halo possible
   for projected features).
 - Phase A projects only own+halo nodes (1/8 of the baseline's replicated
   work); only the tiny per-node dst logits are AllGathered (16B/node,
   bf16 hi/lo pairs).
 - Per-edge dst logits come from a transposed one-hot matmul on the PE
   against an SBUF-resident logit table (no 256B/edge DMA gather).
 - Aggregation per dst block via one-hot matmuls into PSUM; partials
   (payload + denominator hi/lo) land in a [20480, 264] bf16 table split in
   two block-halves; a ReduceScatter(add) per half hands each core its own
   fully-reduced rows. The first RS overlaps the second half of each edge
   phase. Replaces the baseline's 330us serialized feature-AllGather chain.
Outputs (mu, logvar) assembled host-side (un-permuted) from per-core slices.
"""

import numpy as np

# ---- problem constants ----
N = 20000
E = 320000
FIN = 512
HID = 256
LAT = 128
H = 4
C1 = 64
NEG = 0.2
EPS = 1e-16

NC = 8
NOWN = 2500
SEC = 2560               # padded section rows (20 blocks)
NBLK = NC * 20           # 160 global dst blocks
RPB = 125                # real nodes per block
AUG = 3584               # local src table rows (2560 own + 1024 halo)
XW = 384                 # physical row width of gather tables (768B)
CW = 264                 # used row width / partial table width

TPB1 = 2                 # L1 tiles per cell
TILES1 = NBLK * TPB1     # 320
SLOTS1 = TILES1 * 128    # 40960
TPB2 = 3                 # L2 tiles per cell
TILES2 = NBLK * TPB2     # 480
SLOTS2 = TILES2 * 128    # 61440
TPC = 16                 # tiles per chunk
CHUNK = TPC * 128        # 2048
IC = CHUNK // 16         # idx cols per chunk
NCH1 = TILES1 // TPC     # 20
NCH2 = TILES2 // TPC     # 30
NQ = 4                   # ReduceScatter splits (uneven)
QSIZES = (7, 7, 5, 1)    # blocks per quarter; last tiny to shrink the tail
QLO = (0, 7, 14, 19)     # first block of each quarter
QSTART = (0, 56, 112, 152)  # first cell index of each quarter

_cache = {}


def _wrap_idxs(idx):
    n = idx.shape[0]
    t = np.zeros((128, n // 16), np.int16)
    w = idx.reshape(n // 16, 16).T.astype(np.int16)
    for g in range(8):
        t[g * 16:(g + 1) * 16, :] = w
    return t


def _colmajor(a, tiles):
    return np.ascontiguousarray(a.reshape(tiles, 128).T)


def _rowmajor_tiles(a, tiles):
    # per-tile rows for the transposed one-hot build: [128, ceil(T/128), 128]
    reps = (tiles + 127) // 128
    out = np.zeros((128, reps, 128), a.dtype)
    ar = a.reshape(tiles, 128)
    for t in range(tiles):
        out[t % 128, t // 128, :] = ar[t]
    return out


def _build_module(upto=4):
    import concourse.bacc as bacc
    import concourse.mybir as mybir
    import concourse.tile as tile

    f32 = mybir.dt.float32
    bf16 = mybir.dt.bfloat16
    i16 = mybir.dt.int16
    Alu = mybir.AluOpType
    Act = mybir.ActivationFunctionType

    nc = bacc.Bacc("TRN2", target_bir_lowering=False, num_devices=NC,
                   dynamic_dma_scratch_size=65536)

    # ---- inputs ----
    xTb = nc.dram_tensor("xTb", [FIN, AUG], bf16, kind="ExternalInput")
    w1e = nc.dram_tensor("w1e", [FIN, 264], bf16, kind="ExternalInput")
    wmue = nc.dram_tensor("wmue", [HID, 130], bf16, kind="ExternalInput")
    wlve = nc.dram_tensor("wlve", [HID, 130], bf16, kind="ExternalInput")
    b1b = nc.dram_tensor("b1b", [128, 256], f32, kind="ExternalInput")
    bmub = nc.dram_tensor("bmub", [128, 128], f32, kind="ExternalInput")
    blvb = nc.dram_tensor("blvb", [128, 128], f32, kind="ExternalInput")
    iota = nc.dram_tensor("iota", [128, 128], bf16, kind="ExternalInput")
    ident = nc.dram_tensor("ident", [128, 128], f32, kind="ExternalInput")
    srcg1 = nc.dram_tensor("srcg1", [128, SLOTS1 // 16], i16,
                           kind="ExternalInput")
    srcg2 = nc.dram_tensor("srcg2", [128, SLOTS2 // 16], i16,
                           kind="ExternalInput")
    dstoffT1 = nc.dram_tensor("dstoffT1", [128, TILES1], f32,
                              kind="ExternalInput")
    dstoffT2 = nc.dram_tensor("dstoffT2", [128, TILES2], f32,
                              kind="ExternalInput")
    dstl1 = nc.dram_tensor("dstl1", [128, SLOTS1 // 16], i16,
                           kind="ExternalInput")
    dstl2 = nc.dram_tensor("dstl2", [128, SLOTS2 // 16], i16,
                           kind="ExternalInput")
    wT1 = nc.dram_tensor("wT1", [128, TILES1], f32, kind="ExternalInput")
    wT2 = nc.dram_tensor("wT2", [128, TILES2], f32, kind="ExternalInput")

    out2 = nc.dram_tensor("out2", [SEC, 2 * LAT], f32,
                          kind="ExternalOutput")

    with tile.TileContext(nc) as tc:
        with (
            tc.tile_pool(name="cst", bufs=1) as cst,
            tc.tile_pool(name="lw", bufs=2) as lw,
            tc.tile_pool(name="xa", bufs=3) as xa,
            tc.tile_pool(name="gx", bufs=3) as gx,
            tc.tile_pool(name="oh", bufs=44) as ohp,
            tc.tile_pool(name="sm", bufs=8) as sm,
            tc.tile_pool(name="fin", bufs=4) as fin,
            tc.tile_pool(name="ps2", bufs=3, space="PSUM") as ps2,
            tc.tile_pool(name="psa", bufs=1, space="PSUM") as psa,
            tc.tile_pool(name="ps1", bufs=1, space="PSUM") as ps1,
            tc.tile_pool(name="ge", bufs=4) as ge,
            tc.tile_pool(name="dr", bufs=1, space="DRAM") as dr,
        ):
            XPT = dr.tile([AUG, XW], bf16, tag="XPT")
            MLT = dr.tile([SEC, XW], bf16, tag="MLT")
            sd1in = dr.tile([SEC, 8], bf16, tag="sd1in")
            SD1T = dr.tile([NC * SEC // 16, 128], bf16, tag="SD1T",
                           addr_space="Shared")
            SDW1 = dr.tile([NC * SEC, 128], bf16, tag="SDW1")
            sd2in = dr.tile([SEC, 8], bf16, tag="sd2in")
            SD2T = dr.tile([NC * SEC // 16, 128], bf16, tag="SD2T",
                           addr_space="Shared")
            SDW2 = dr.tile([NC * SEC, 128], bf16, tag="SDW2")
            TBL1 = []
            TBL2 = []
            RS1O = []
            RS2O = []
            for hf in range(NQ):
                qr = QSIZES[hf] * 128
                TBL1.append(dr.tile([NC * qr, CW], bf16, tag=f"TBL1{hf}",
                                    name=f"TBL1{hf}"))
                TBL2.append(dr.tile([NC * qr, CW], bf16, tag=f"TBL2{hf}",
                                    name=f"TBL2{hf}"))
                RS1O.append(dr.tile([qr, CW], bf16, tag=f"RS1O{hf}",
                                    name=f"RS1O{hf}"))
                RS2O.append(dr.tile([qr, CW], bf16, tag=f"RS2O{hf}",
                                    name=f"RS2O{hf}"))

            # resident constants
            def cload(shape, dtype, tag, srcap):
                t = cst.tile(shape, dtype, tag=tag)
                nc.sync.dma_start(t[:], srcap)
                return t

            w1e_t = [cload([128, 264], bf16, f"w1e{kk}",
                           w1e[kk * 128:(kk + 1) * 128, :]) for kk in range(4)]

            # ---- phase A: own groups, then AG1, then halo group ----
            def phase_a_group(g):
                lx = lw.tile([128, 4, 512], bf16, tag="lx")
                nc.sync.dma_start(
                    lx[:], xTb[:].rearrange("(kk p) (g n) -> p kk g n",
                                            p=128, n=512)[:, :, g, :])
                xps = xa.tile([128, 4, 272], bf16, tag="xps")
                sdh = xa.tile([128, 4, 8], bf16, tag="sdh")
                for ti in range(4):
                    ps = psa.tile([128, 264], f32, tag="psA", name="psA",
                                  bufs=2)
                    for kk in range(4):
                        sl = slice(ti * 128, (ti + 1) * 128)
                        nc.tensor.matmul(ps[:], lx[:, kk, sl], w1e_t[kk][:],
                                         start=(kk == 0), stop=(kk == 3))
                    nc.scalar.copy(xps[:, ti, 0:256], ps[:, 0:256])
                    # ss as f32 in slots 256:264
                    nc.vector.tensor_copy(
                        xps[:, ti, 256:272].bitcast(f32), ps[:, 256:264])
                    # sd hi/lo bf16 pairs for the logit AllGather
                    nc.vector.tensor_copy(sdh[:, ti, 0:4], ps[:, 260:264])
                    nc.vector.tensor_tensor(
                        sdh[:, ti, 4:8], ps[:, 260:264], sdh[:, ti, 0:4],
                        op=Alu.subtract)
                nc.sync.dma_start(
                    XPT[:].rearrange("(g4 p) c -> p g4 c", p=128)
                    [:, 4 * g:4 * g + 4, 0:264], xps[:, :, 0:264])
                if g < SEC // 512:
                    nc.sync.dma_start(
                        sd1in[:].rearrange("(g4 p) c -> p g4 c", p=128)
                        [:, 4 * g:4 * g + 4, :], sdh[:])

            nc.gpsimd.collective_compute(
                "AllGather", mybir.AluOpType.bypass,
                replica_groups=[list(range(NC))],
                ins=[sd1in[:]], outs=[SD1T[:]])
            for g in range(2):
                phase_a_group(g)
            for s16 in range(16):
                nc.scalar.dma_start(
                    SDW1[:].rearrange("(r s) c -> r s c", s=16)
                    [:, s16, 0:8],
                    SD1T[:, s16 * 8:(s16 + 1) * 8])
            for g in range(2, AUG // 512):
                phase_a_group(g)
            srcg1_t = cload([128, SLOTS1 // 16], i16, "srcg1", srcg1[:])
            dstl1_t = cload([128, SLOTS1 // 16], i16, "dstl1", dstl1[:])
            dstoffT1_t = cload([128, TILES1], f32, "dstoffT1", dstoffT1[:])
            wT1_t = cload([128, TILES1], f32, "wT1", wT1[:])
            iota_t = cload([128, 128], bf16, "iota", iota[:])
            wmue_t = [cload([128, 130], bf16, f"wmue{kk}",
                            wmue[kk * 128:(kk + 1) * 128, :])
                      for kk in range(2)]
            wlve_t = [cload([128, 130], bf16, f"wlve{kk}",
                            wlve[kk * 128:(kk + 1) * 128, :])
                      for kk in range(2)]
            b1b_t = cload([128, 256], f32, "b1b", b1b[:])
            bmub_t = cload([128, 128], f32, "bmub", bmub[:])
            blvb_t = cload([128, 128], f32, "blvb", blvb[:])
            ident_t = cload([128, 128], f32, "ident", ident[:])

            def edge_phase(layer, SRC_TBL, SDT_TBL, srcg_t, dofT, dstl_t,
                           wTt, TBLh, nchunk, tpb, rs_cb):
                nh = 4 if layer == 1 else 2
                blk_ps = {}
                ext = None
                qe = [min((QSTART[q + 1] * tpb + TPC - 1) // TPC + 4,
                          nchunk - 1) for q in range(3)]
                for ci in range(nchunk):
                    for q in range(3):
                        if ci == qe[q]:
                            rs_cb(q)
                    ohx_t = {}
                    for tt in range(TPC):
                        t = ci * TPC + tt
                        ohx = ohp.tile([128, 128], bf16, tag="ohx")
                        eng = nc.vector if tt % 4 != 3 else nc.gpsimd
                        eng.tensor_scalar(
                            ohx[:], iota_t[:], dofT[:, t:t + 1], None,
                            Alu.is_equal)
                        ohx_t[tt] = ohx
                    xrow = gx.tile([128, TPC, XW], bf16, tag="xrow")
                    HT = TPC // 2
                    HIC = IC // 2
                    for gh in range(2):
                        nc.gpsimd.dma_gather(
                            xrow[:, gh * HT:(gh + 1) * HT, :], SRC_TBL[:],
                            srcg_t[:, ci * IC + gh * HIC:
                                   ci * IC + (gh + 1) * HIC],
                            CHUNK // 2, CHUNK // 2, XW)
                    ext = ge.tile([128, TPC, 128], bf16, tag="ext")
                    for gh in range(2):
                        nc.gpsimd.dma_gather(
                            ext[:, gh * HT:(gh + 1) * HT, :], SDT_TBL[:],
                            dstl_t[:, ci * IC + gh * HIC:
                                   ci * IC + (gh + 1) * HIC],
                            CHUNK // 2, CHUNK // 2, 128)
                    exs = ext[:, :, 0:8]
                    z = sm.tile([128, TPC, nh], f32, tag="z")
                    if layer == 1:
                        nc.vector.tensor_tensor(
                            z[:], xrow[:, :, 256:264].bitcast(f32),
                            exs[:, :, 0:4], op=Alu.add)
                        nc.vector.tensor_tensor(
                            z[:], z[:], exs[:, :, 4:8], op=Alu.add)
                    else:
                        nc.vector.tensor_tensor(
                            z[:],
                            xrow[:, :, 256:264].bitcast(f32)[:, :, 0:2],
                            exs[:, :, 0:2], op=Alu.add)
                        nc.vector.tensor_tensor(
                            z[:], z[:], exs[:, :, 4:6], op=Alu.add)
                    nc.vector.scalar_tensor_tensor(
                        z[:], in0=z[:], scalar=NEG, in1=z[:],
                        op0=Alu.mult, op1=Alu.max)
                    ex = sm.tile([128, TPC, nh], f32, tag="ex")
                    nc.scalar.activation(ex[:], z[:], Act.Exp)
                    exw = sm.tile([128, TPC, nh], f32, tag="exw")
                    wb = wTt[:, ci * TPC:(ci + 1) * TPC]
                    nc.vector.tensor_tensor(
                        exw[:], ex[:],
                        wb.rearrange("p (t o) -> p t o", o=1).to_broadcast(
                            [128, TPC, nh]), op=Alu.mult)
                    exw2 = sm.tile([128, TPC, nh, 2], bf16, tag="exw2")
                    nc.vector.tensor_copy(
                        exw2[:], exw[:].rearrange("p t (h o) -> p t h o", o=1)
                        .to_broadcast([128, TPC, nh, 2]))
                    kw = 256 // nh // 2
                    xrh = xrow[:, :, 0:256].rearrange(
                        "p t (h k two) -> p t h k two", h=nh, two=2)
                    nc.vector.tensor_tensor(
                        xrh, xrh,
                        exw2[:].rearrange("p t h (o two) -> p t h o two",
                                          two=2)
                        .to_broadcast([128, TPC, nh, kw, 2]), op=Alu.mult)
                    nc.vector.tensor_copy(xrow[:, :, 256:256 + nh], ex[:])
                    nc.vector.tensor_tensor(
                        xrow[:, :, 256 + nh:256 + 2 * nh], ex[:],
                        xrow[:, :, 256:256 + nh], op=Alu.subtract)

                    for tt in range(TPC):
                        t = ci * TPC + tt
                        cell = t // tpb
                        k = t % tpb
                        if k == 0:
                            blk_ps[cell] = ps2.tile([128, 264], f32,
                                                    tag="blk", name="blkps",
                                                    bufs=4)
                        ps = blk_ps[cell]
                        nc.tensor.matmul(
                            ps[:, 0:264], ohx_t[tt][:], xrow[:, tt, 0:264],
                            start=(k == 0), stop=(k == tpb - 1))
                        if k == tpb - 1:
                            hf = max(q for q in range(NQ)
                                     if QSTART[q] <= cell)
                            rb = cell - QSTART[hf]
                            cpy = fin.tile([128, 264], bf16, tag="cpy",
                                           bufs=8)
                            nc.scalar.copy(cpy[:], ps[:, 0:264])
                            nc.sync.dma_start(
                                TBLh[hf][rb * 128:(rb + 1) * 128, :], cpy[:])
                            del blk_ps[cell]

            def emit_rs(TBLh, RSOh, hf):
                nc.gpsimd.collective_compute(
                    "ReduceScatter", Alu.add,
                    replica_groups=[list(range(NC))],
                    ins=[TBLh[hf][:]], outs=[RSOh[hf][:]])

            srcg2_t = cload([128, SLOTS2 // 16], i16, "srcg2", srcg2[:])
            dstoffT2_t = cload([128, TILES2], f32, "dstoffT2", dstoffT2[:])
            dstl2_t = cload([128, SLOTS2 // 16], i16, "dstl2", dstl2[:])
            wT2_t = cload([128, TILES2], f32, "wT2", wT2[:])
            # ---- L1 ----
            if upto >= 2:
                edge_phase(1, XPT, SDW1, srcg1_t, dstoffT1_t, dstl1_t, wT1_t,
                           TBL1, NCH1, TPB1,
                           (lambda hf: emit_rs(TBL1, RS1O, hf))
                           if upto >= 3 else (lambda hf: None))
            if upto >= 3:
                emit_rs(TBL1, RS1O, 3)

            # ---- L1 finalize ----
            for b in range(20 if upto >= 3 else 0):
                hf = max(q for q in range(NQ) if QLO[q] <= b)
                rbase = (b - QLO[hf]) * 128
                rsb = fin.tile([128, 264], bf16, tag="rsb")
                nc.sync.dma_start(rsb[:], RS1O[hf][rbase:rbase + 128, :])
                den = sm.tile([128, 4], f32, tag="den")
                nc.vector.tensor_tensor(den[:], rsb[:, 256:260],
                                        rsb[:, 260:264], op=Alu.add)
                nc.vector.tensor_scalar_add(den[:], den[:], EPS)
                rec = sm.tile([128, 4], f32, tag="rec")
                nc.vector.reciprocal(rec[:], den[:])
                hb = fin.tile([128, 256], f32, tag="hb")
                for h in range(H):
                    nc.vector.scalar_tensor_tensor(
                        hb[:, h * 64:(h + 1) * 64],
                        in0=rsb[:, h * 64:(h + 1) * 64],
                        scalar=rec[:, h:h + 1],
                        in1=b1b_t[:, h * 64:(h + 1) * 64],
                        op0=Alu.mult, op1=Alu.add)
                zm = fin.tile([128, 256], f32, tag="zm")
                nc.vector.tensor_scalar_min(zm[:], hb[:], 0.0)
                ez = fin.tile([128, 256], f32, tag="ez")
                nc.scalar.activation(ez[:], zm[:], Act.Exp)
                nc.vector.scalar_tensor_tensor(
                    hb[:], in0=hb[:], scalar=0.0, in1=ez[:],
                    op0=Alu.max, op1=Alu.add)
                nc.vector.tensor_scalar_add(hb[:], hb[:], -1.0)
                hTs = []
                for half in range(2):
                    pst = psa.tile([128, 264], f32, tag="psA", name="pstA",
                                   bufs=2)
                    nc.tensor.transpose(
                        pst[:, 0:128], hb[:, half * 128:(half + 1) * 128],
                        ident_t[:])
                    hT = fin.tile([128, 128], bf16, tag=f"hT{half}")
                    nc.vector.tensor_copy(hT[:], pst[:, 0:128])
                    hTs.append(hT)
                psmu_t = ps1.tile([128, 130], f32, tag="psmu", name="psmu")
                pslv_t = ps1.tile([128, 130], f32, tag="pslv", name="pslv")
                psmu = psmu_t[:]
                pslv = pslv_t[:]
                for kk in range(2):
                    nc.tensor.matmul(psmu, hTs[kk][:], wmue_t[kk][:],
                                     start=(kk == 0), stop=(kk == 1))
                    nc.tensor.matmul(pslv, hTs[kk][:], wlve_t[kk][:],
                                     start=(kk == 0), stop=(kk == 1))
                xr2 = fin.tile([128, 264], bf16, tag="xr2")
                nc.scalar.copy(xr2[:, 0:128], psmu[:, 0:128])
                nc.scalar.copy(xr2[:, 128:256], pslv[:, 0:128])
                # ss as f32 slots [ssmu, sslv]; sd hi/lo bf16 for AG2
                ssv = xr2[:, 256:264].bitcast(f32)
                nc.vector.tensor_copy(ssv[:, 0:1], psmu[:, 128:129])
                nc.vector.tensor_copy(ssv[:, 1:2], pslv[:, 128:129])
                sdh = fin.tile([128, 8], bf16, tag="sdh2")
                nc.vector.tensor_copy(sdh[:, 0:1], psmu[:, 129:130])
                nc.vector.tensor_copy(sdh[:, 1:2], pslv[:, 129:130])
                nc.vector.tensor_tensor(sdh[:, 4:5], psmu[:, 129:130],
                                        sdh[:, 0:1], op=Alu.subtract)
                nc.vector.tensor_tensor(sdh[:, 5:6], pslv[:, 129:130],
                                        sdh[:, 1:2], op=Alu.subtract)
                nc.sync.dma_start(MLT[b * 128:(b + 1) * 128, 0:264], xr2[:])
                nc.sync.dma_start(sd2in[b * 128:(b + 1) * 128, :], sdh[:])

            if upto >= 3:
                nc.gpsimd.collective_compute(
                    "AllGather", mybir.AluOpType.bypass,
                    replica_groups=[list(range(NC))],
                    ins=[sd2in[:]], outs=[SD2T[:]])
                for s16 in range(16):
                    nc.scalar.dma_start(
                        SDW2[:].rearrange("(r s) c -> r s c", s=16)
                        [:, s16, 0:8],
                        SD2T[:, s16 * 8:(s16 + 1) * 8])

            # ---- L2/3 ----
            if upto >= 4:
                edge_phase(2, MLT, SDW2, srcg2_t, dstoffT2_t, dstl2_t, wT2_t,
                           TBL2, NCH2, TPB2,
                           lambda hf: emit_rs(TBL2, RS2O, hf))
                emit_rs(TBL2, RS2O, 3)

            # ---- final ----
            for b in range(20 if upto >= 4 else 1):
                hf = max(q for q in range(NQ) if QLO[q] <= b)
                rbase = (b - QLO[hf]) * 128
                rsb = fin.tile([128, 264], bf16, tag="rsb2")
                nc.sync.dma_start(rsb[:], RS2O[hf][rbase:rbase + 128, :])
                ob = fin.tile([128, 256], f32, tag="ob")
                for li, bias_t in enumerate((bmub_t, blvb_t)):
                    den = sm.tile([128, 1], f32, tag="den2")
                    nc.vector.tensor_tensor(
                        den[:], rsb[:, 256 + li:257 + li],
                        rsb[:, 258 + li:259 + li], op=Alu.add)
                    nc.vector.tensor_scalar_add(den[:], den[:], EPS)
                    rec = sm.tile([128, 1], f32, tag="rec2")
                    nc.vector.reciprocal(rec[:], den[:])
                    nc.vector.scalar_tensor_tensor(
                        ob[:, li * 128:(li + 1) * 128],
                        in0=rsb[:, li * 128:(li + 1) * 128],
                        scalar=rec[:, 0:1],
                        in1=bias_t[:], op0=Alu.mult, op1=Alu.add)
                nc.sync.dma_start(
                    out2[b * 128:(b + 1) * 128, :], ob[:])

    nc.compile()
    return nc


def _prep_inputs(x, edge_index, edge_weight, W1, att1, b1, Wmu, attmu, bmu,
                 Wlv, attlv, blv):
    import ml_dtypes
    bf = ml_dtypes.bfloat16

    src = np.asarray(edge_index[0], np.int64)
    dst = np.asarray(edge_index[1], np.int64)
    w = np.asarray(edge_weight, np.float32)
    x = np.asarray(x, np.float32)

    att1 = np.asarray(att1, np.float32)
    W1 = np.asarray(W1, np.float32)
    Wss1 = np.zeros((FIN, H), np.float32)
    Wsd1 = np.zeros((FIN, H), np.float32)
    for h in range(H):
        Wss1[:, h] = W1[:, h * C1:(h + 1) * C1] @ att1[h, C1:]
        Wsd1[:, h] = W1[:, h * C1:(h + 1) * C1] @ att1[h, :C1]
    w1e = np.concatenate([W1, Wss1, Wsd1], axis=1).astype(bf)

    attmu = np.asarray(attmu, np.float32).reshape(-1)
    attlv = np.asarray(attlv, np.float32).reshape(-1)
    Wmu = np.asarray(Wmu, np.float32)
    Wlv = np.asarray(Wlv, np.float32)
    wmue = np.concatenate(
        [Wmu, (Wmu @ attmu[LAT:])[:, None], (Wmu @ attmu[:LAT])[:, None]],
        axis=1).astype(bf)
    wlve = np.concatenate(
        [Wlv, (Wlv @ attlv[LAT:])[:, None], (Wlv @ attlv[:LAT])[:, None]],
        axis=1).astype(bf)

    xT_all = x.T.astype(bf)
    b1b = np.tile(np.asarray(b1, np.float32)[None, :], (128, 1))
    bmub = np.tile(np.asarray(bmu, np.float32)[None, :], (128, 1))
    blvb = np.tile(np.asarray(blv, np.float32)[None, :], (128, 1))
    iota = np.tile(np.arange(128, dtype=np.float32)[None, :],
                   (128, 1)).astype(bf)
    iotaP = np.ascontiguousarray(iota.T)
    ident = np.eye(128, dtype=np.float32)

    # ---- node permutation: LPT balance blocks by in-degree ----
    deg = np.bincount(dst, minlength=N).astype(np.int64)
    permrow = np.zeros(N, np.int64)     # node -> global padded row
    inv_rows = np.full(NC * SEC, -1, np.int64)  # row -> node
    for c in range(NC):
        nodes = np.arange(c * NOWN, (c + 1) * NOWN)
        order = nodes[np.argsort(-deg[nodes], kind="stable")]
        loads = np.zeros(20, np.int64)
        fill = np.zeros(20, np.int64)
        for nd in order:
            cand = np.where(fill < RPB)[0]
            bsel = cand[np.argmin(loads[cand])]
            permrow[nd] = c * SEC + bsel * 128 + fill[bsel]
            inv_rows[permrow[nd]] = nd
            loads[bsel] += deg[nd]
            fill[bsel] += 1

    prow = permrow[dst]
    gblk = prow // 128
    goff = prow % 128
    blktot = np.bincount(gblk, minlength=NBLK)
    assert blktot.max() <= NC * 256, f"block overflow {blktot.max()}"

    score = src // NOWN
    # L1 cell rebalancing: cap 256 per (core, block), move with halo
    assign = score.copy()
    eidx_by_cell = {}
    for b in range(NBLK):
        eb = np.where(gblk == b)[0]
        cores = assign[eb]
        cnt = np.bincount(cores, minlength=NC)
        over_c = [c for c in range(NC) if cnt[c] > 256]
        space = {c: 256 - int(cnt[c]) for c in range(NC)}
        movers = []
        for c in over_c:
            ec = eb[cores == c]
            movers.extend(ec[256:].tolist())
            space[c] = 0
        ptr = 0
        order = sorted(space, key=lambda k: -space[k])
        while ptr < len(movers):
            moved = False
            for c in order:
                if space[c] > 0 and ptr < len(movers):
                    assign[movers[ptr]] = c
                    space[c] -= 1
                    ptr += 1
                    moved = True
            assert moved
        assert ptr == len(movers)

    # local row maps: own nodes at their permuted slot, halo appended
    QSIZES = (7, 7, 5, 1)
    QLO = (0, 7, 14, 19)
    QSTART = (0, 56, 112, 152)
    lb = gblk % 20
    q_of = np.select([lb >= 19, lb >= 14, lb >= 7], [3, 2, 1], 0)
    qlo_a = np.array(QLO)[q_of]
    qsz_a = np.array(QSIZES)[q_of]
    qst_a = np.array(QSTART)[q_of]
    cell_of = qst_a + (gblk // 20) * qsz_a + (lb - qlo_a)

    in_maps = []
    for c in range(NC):
        m1 = assign == c
        e1 = np.where(m1)[0]
        own = score[e1] == c
        halo_nodes = np.unique(src[e1[~own]])
        assert SEC + len(halo_nodes) <= AUG, len(halo_nodes)
        lrow = np.full(N, -1, np.int64)
        sec_nodes = inv_rows[c * SEC:(c + 1) * SEC]
        valid = sec_nodes >= 0
        lrow[sec_nodes[valid]] = np.where(valid)[0]
        lrow[halo_nodes] = SEC + np.arange(len(halo_nodes))

        def pack(eids, tpb, tiles, slots):
            e_src = np.zeros(slots, np.int64)
            e_dst = np.zeros(slots, np.int64)
            e_off = np.full(slots, -1.0, np.float32)
            e_w = np.zeros(slots, np.float32)
            cells = cell_of[eids]
            for cell in range(160):
                bm = cells == cell
                ee = eids[bm]
                nbe = len(ee)
                assert nbe <= tpb * 128, (c, cell, nbe)
                o = cell * tpb * 128
                e_src[o:o + nbe] = lrow[src[ee]]
                e_dst[o:o + nbe] = prow[ee]
                e_off[o:o + nbe] = goff[ee].astype(np.float32)
                e_w[o:o + nbe] = w[ee]
            assert e_src.min() >= 0
            return e_src, e_dst, e_off, e_w

        s1, d1, o1, w1 = pack(e1, TPB1, TILES1, SLOTS1)
        e2 = np.where(score == c)[0]
        s2, d2, o2, w2 = pack(e2, TPB2, TILES2, SLOTS2)

        xTb_c = np.zeros((FIN, AUG), bf)
        cols = sec_nodes.copy()
        ok = cols >= 0
        xTb_c[:, :SEC][:, ok] = xT_all[:, cols[ok]]
        xTb_c[:, SEC:SEC + len(halo_nodes)] = xT_all[:, halo_nodes]

        in_maps.append({
            "xTb": xTb_c, "w1e": w1e, "wmue": wmue, "wlve": wlve, "b1b": b1b,
            "bmub": bmub, "blvb": blvb, "iota": iota, "iotaP": iotaP,
            "ident": ident,
            "srcg1": _wrap_idxs(s1), "srcg2": _wrap_idxs(s2),
            "dstoffT1": _colmajor(o1, TILES1),
            "dstoffT2": _colmajor(o2, TILES2),
            "dstl1": _wrap_idxs(d1), "dstl2": _wrap_idxs(d2),
            "wT1": _colmajor(w1, TILES1), "wT2": _colmajor(w2, TILES2),
        })
    return in_maps, inv_rows


def kernel(x, edge_index, edge_weight, W1, att1, b1, Wmu, attmu, bmu,
           Wlv, attlv, blv):
    from concourse.bass_utils import run_bass_kernel_spmd

    if "nc" not in _cache:
        _cache["nc"] = _build_module()
    nc = _cache["nc"]
    in_maps, inv_rows = _prep_inputs(x, edge_index, edge_weight, W1, att1, b1,
                                     Wmu, attmu, bmu, Wlv, attlv, blv)
    r = run_bass_kernel_spmd(nc, in_maps, list(range(NC)))
    mu = np.zeros((N, LAT), np.float32)
    lv = np.zeros((N, LAT), np.float32)
    for c in range(NC):
        rows = inv_rows[c * SEC:(c + 1) * SEC]
        ok = rows >= 0
        o2 = r.results[c]["out2"]
        mu[rows[ok]] = o2[ok, 0:LAT]
        lv[rows[ok]] = o2[ok, LAT:]
    return (mu, lv)
